# revision 1
# baseline (speedup 1.0000x reference)
"""GATv2 (2-layer, PyG semantics) on 8 Trainium2 NeuronCores.

Strategy (graph/data parallel, dst-sharded):
  - Nodes sharded by destination range across 8 cores (12500 nodes/core).
  - Each core builds the full layer-1 feature table [xl|xr] (fp16) from a
    host-transposed x, gathers xl[src]/xr[dst] per edge with indirect DMA,
    does the edge softmax math on DVE/ACT, and aggregates num/den per
    128-node destination window with one-hot matmuls accumulated in PSUM.
  - The window epilogue produces h^T (feature-major), which feeds the
    layer-2 dense matmul as the stationary operand (no transposes).
  - Local layer-2 tables are AllGathered; the edge phase repeats for
    layer 2; raw [num|den] go back to the host which finishes the
    divide/mean/bias and the final transpose.
"""

import functools
import sys

import numpy as np

sys.path.insert(0, "/opt/trn_rl_repo")

# ---------------------------------------------------------------- constants
N = 100_000
E = 1_600_000
IN = 9
HID = 16
H1 = 4
H2 = 4
OUT = 9
D1 = H1 * HID  # 64
D2 = H2 * OUT  # 36
NEG_ATT = 0.2
NEG_ACT = 0.01
NCORES = 8
NSH = N // NCORES  # 12500 nodes per core
WIN = 128  # dst nodes per window
P = 128


class Cfg:
    """Compile-time geometry. Full-size defaults; overridable for sim tests."""

    def __init__(self, n=N, e=E, ncores=NCORES, tw=18, dense_chunks=16):
        self.n = n
        self.e = e
        self.ncores = ncores
        self.nsh = n // ncores
        self.nw = -(-self.nsh // WIN)  # windows per core
        self.tw = tw  # tiles (of 128 edges) per window, static
        self.tt = self.nw * tw  # total tiles per core
        self.rows1 = ((n + 1023) // 1024) * 1024  # padded node rows, table1
        self.rows2sh = self.nw * WIN  # padded rows per core, table2 shard
        self.rows2 = self.rows2sh * ncores
        self.dense_tiles = self.rows1 // P
        self.dense_chunks = dense_chunks
        assert self.dense_tiles % dense_chunks == 0
        self.chunk_tiles = self.dense_tiles // dense_chunks
        # table1 write batching (tiles per DMA)
        self.wb = 7 if self.chunk_tiles % 7 == 0 else 1
        assert self.chunk_tiles % self.wb == 0


CFG = Cfg()

F16 = None  # set lazily (mybir import)


# ---------------------------------------------------------------- device code
def build_program(cfg: Cfg):
    """Build the SPMD single-core Bass program (same NEFF on all cores)."""
    import concourse.bacc as bacc
    import concourse.bass as bass
    import concourse.tile as tile
    from concourse import mybir

    f32 = mybir.dt.float32
    f16 = mybir.dt.float16
    i32 = mybir.dt.int32
    AF = mybir.ActivationFunctionType
    ALU = mybir.AluOpType

    nc = bacc.Bacc("TRN2", target_bir_lowering=False, debug=False,
                   num_devices=cfg.ncores)

    TW, NW, TT = cfg.tw, cfg.nw, cfg.tt

    # ---------------- dram I/O
    xT = nc.dram_tensor("xT", [IN + 1, cfg.rows1], f16, kind="ExternalInput")
    w1c = nc.dram_tensor("w1c", [IN + 1, 2 * D1], f16, kind="ExternalInput")
    w2c = nc.dram_tensor("w2c", [D1, 2 * D2], f16, kind="ExternalInput")
    b2c = nc.dram_tensor("b2c", [1, 2 * D2], f32, kind="ExternalInput")
    att1 = nc.dram_tensor("att1", [1, D1], f16, kind="ExternalInput")
    att2 = nc.dram_tensor("att2", [1, D2], f16, kind="ExternalInput")
    bias1 = nc.dram_tensor("bias1", [D1, 1], f32, kind="ExternalInput")
    e4 = nc.dram_tensor("e4", [H1, D1], f32, kind="ExternalInput")
    srcA = nc.dram_tensor("srcA", [P, TT], i32, kind="ExternalInput")
    dstA = nc.dram_tensor("dstA", [P, TT], i32, kind="ExternalInput")
    srcB = nc.dram_tensor("srcB", [P, TT], i32, kind="ExternalInput")
    dstB = nc.dram_tensor("dstB", [P, TT], i32, kind="ExternalInput")
    dstoff = nc.dram_tensor("dstoff", [P, TT], f32, kind="ExternalInput")
    out_raw = nc.dram_tensor("out_raw", [D2 + H2, cfg.rows2sh], f32,
                             kind="ExternalOutput")

    GW = 64  # gather row width (fp16, 128B rows) for all tables
    t1l = nc.dram_tensor("t1l", [cfg.rows1, GW], f16)
    t1r = nc.dram_tensor("t1r", [cfg.rows1, GW], f16)
    t2l = nc.dram_tensor("t2l", [cfg.rows2sh, GW], f16)
    t2r = nc.dram_tensor("t2r", [cfg.rows2sh, GW], f16)
    t2lg = nc.dram_tensor("t2lg", [cfg.rows2, GW], f16, addr_space="Shared")
    t2rg = nc.dram_tensor("t2rg", [cfg.rows2, GW], f16, addr_space="Shared")

    OC1 = D1 + H1  # agg cols layer 1 (num 64 + den 4)
    OC2 = D2 + H2  # agg cols layer 2 (num 36 + den 4)

    with tile.TileContext(nc) as tc:
        import contextlib
        ctx = contextlib.ExitStack()
        with ctx:
            consts = ctx.enter_context(tc.tile_pool(name="consts", bufs=1))
            idxp = ctx.enter_context(tc.tile_pool(name="idxp", bufs=1))
            xtp = ctx.enter_context(tc.tile_pool(name="xtp", bufs=2))
            stage = ctx.enter_context(tc.tile_pool(name="stage", bufs=3))
            gath = ctx.enter_context(tc.tile_pool(name="gath", bufs=3))
            emath = ctx.enter_context(tc.tile_pool(name="emath", bufs=2))
            ohp = ctx.enter_context(tc.tile_pool(name="ohp", bufs=2))
            wtp = ctx.enter_context(tc.tile_pool(name="wtp", bufs=2))
            epi = ctx.enter_context(tc.tile_pool(name="epi", bufs=2))
            psum = ctx.enter_context(tc.tile_pool(name="psum", bufs=2,
                                                  space="PSUM"))
            psd = ctx.enter_context(tc.tile_pool(name="psd", bufs=2,
                                                 space="PSUM"))

            # ---------------- constants into SBUF
            w1c_sb = consts.tile([IN + 1, 2 * D1], f16)
            nc.sync.dma_start(out=w1c_sb[:], in_=w1c.ap())
            w2c_sb = consts.tile([D1, 2 * D2], f16)
            nc.sync.dma_start(out=w2c_sb[:], in_=w2c.ap())
            bias1_sb = consts.tile([D1, 1], f32)
            nc.sync.dma_start(out=bias1_sb[:], in_=bias1.ap())
            e4_sb = consts.tile([H1, D1], f32)
            nc.sync.dma_start(out=e4_sb[:], in_=e4.ap())
            att1_sb = consts.tile([P, D1], f16)
            nc.sync.dma_start(out=att1_sb[0:1, :], in_=att1.ap())
            nc.gpsimd.partition_broadcast(att1_sb[:], att1_sb[0:1, :])
            att2_sb = consts.tile([P, D2], f16)
            nc.sync.dma_start(out=att2_sb[0:1, :], in_=att2.ap())
            nc.gpsimd.partition_broadcast(att2_sb[:], att2_sb[0:1, :])
            b2rep = consts.tile([P, 2 * D2], f32)
            nc.sync.dma_start(out=b2rep[0:1, :], in_=b2c.ap())
            nc.gpsimd.partition_broadcast(b2rep[:], b2rep[0:1, :])
            iota_i = consts.tile([P, P], i32)
            nc.gpsimd.iota(iota_i[:], pattern=[[1, P]], base=0,
                           channel_multiplier=0)
            iota_f = consts.tile([P, P], f16)
            nc.vector.tensor_copy(out=iota_f[:], in_=iota_i[:])
            eps4 = consts.tile([H1, 1], f32)
            nc.vector.memset(eps4[:], 1e-16)

            # idx arrays
            srcA_sb = idxp.tile([P, TT], i32)
            nc.sync.dma_start(out=srcA_sb[:], in_=srcA.ap())
            dstA_sb = idxp.tile([P, TT], i32)
            nc.sync.dma_start(out=dstA_sb[:], in_=dstA.ap())
            srcB_sb = idxp.tile([P, TT], i32)
            nc.sync.dma_start(out=srcB_sb[:], in_=srcB.ap())
            dstB_sb = idxp.tile([P, TT], i32)
            nc.sync.dma_start(out=dstB_sb[:], in_=dstB.ap())
            doff_sb = idxp.tile([P, TT], f32)
            nc.sync.dma_start(out=doff_sb[:], in_=dstoff.ap())

            # ---------------- phase 1: dense layer-1 table
            ck = cfg.chunk_tiles  # tiles per xT chunk
            t1lv = t1l.ap().rearrange("(b t n) f -> b n t f", t=cfg.wb, n=P)
            t1rv = t1r.ap().rearrange("(b t n) f -> b n t f", t=cfg.wb, n=P)
            for c in range(cfg.dense_chunks):
                xt_sb = xtp.tile([IN + 1, ck * P], f16)
                nc.sync.dma_start(out=xt_sb[:],
                                  in_=xT.ap()[:, c * ck * P:(c + 1) * ck * P])
                for b in range(ck // cfg.wb):
                    st = stage.tile([P, cfg.wb, 2 * D1], f16)
                    for j in range(cfg.wb):
                        t = b * cfg.wb + j
                        mm = psd.tile([P, 2 * D1], f32)
                        nc.tensor.matmul(out=mm[:],
                                         lhsT=xt_sb[:, t * P:(t + 1) * P],
                                         rhs=w1c_sb[:], start=True, stop=True)
                        if j % 2 == 0:
                            nc.scalar.copy(out=st[:, j, :], in_=mm[:])
                        else:
                            nc.vector.tensor_copy(out=st[:, j, :], in_=mm[:])
                    gb = c * (ck // cfg.wb) + b
                    nc.sync.dma_start(out=t1lv[gb], in_=st[:, :, 0:D1])
                    nc.sync.dma_start(out=t1rv[gb], in_=st[:, :, D1:2 * D1])

            tc.strict_bb_all_engine_barrier()

            # ---------------- edge phase builder (shared by both layers)
            def edge_layer(layer):
                if layer == 1:
                    D, H, C, OC = D1, H1, HID, OC1
                    tl, tr, s_idx, d_idx = t1l, t1r, srcA_sb, dstA_sb
                    att_sb = att1_sb
                else:
                    D, H, C, OC = D2, H2, OUT, OC2
                    tl, tr, s_idx, d_idx = t2lg, t2rg, srcB_sb, dstB_sb
                    att_sb = att2_sb

                for w in range(NW):
                    ts, te = w * TW, (w + 1) * TW
                    xl_g = gath.tile([P, TW, GW], f16, tag="xl")
                    xr_g = gath.tile([P, TW, GW], f16, tag="xr")
                    for t in range(TW):
                        nc.gpsimd.indirect_dma_start(
                            out=xl_g[:, t, :], out_offset=None, in_=tl.ap(),
                            in_offset=bass.IndirectOffsetOnAxis(
                                ap=s_idx[:, ts + t:ts + t + 1], axis=0),
                            element_offset=0)
                        nc.gpsimd.indirect_dma_start(
                            out=xr_g[:, t, :], out_offset=None, in_=tr.ap(),
                            in_offset=bass.IndirectOffsetOnAxis(
                                ap=d_idx[:, ts + t:ts + t + 1], axis=0),
                            element_offset=0)

                    oh = ohp.tile([P, TW, P], f16, tag="oh")
                    for t in range(TW):
                        nc.vector.tensor_scalar(
                            oh[:, t, :], iota_f[:],
                            doff_sb[:, ts + t:ts + t + 1], None,
                            op0=ALU.is_equal)

                    epre = emath.tile([P, TW, D], f16, tag="epre")
                    nc.vector.tensor_tensor(out=epre[:], in0=xl_g[:, :, 0:D],
                                            in1=xr_g[:, :, 0:D], op=ALU.add)
                    ee = emath.tile([P, TW, D], f16, tag="ee")
                    nc.vector.scalar_tensor_tensor(
                        out=ee[:], in0=epre[:], scalar=NEG_ATT, in1=epre[:],
                        op0=ALU.mult, op1=ALU.max)
                    tmp = emath.tile([P, TW, D], f16, tag="tmp")
                    nc.vector.tensor_tensor(
                        out=tmp[:], in0=ee[:],
                        in1=att_sb[:].unsqueeze(1).to_broadcast([P, TW, D]),
                        op=ALU.mult)
                    logits = emath.tile([P, TW * H], f32, tag="logits")
                    nc.vector.tensor_reduce(
                        out=logits[:],
                        in_=tmp[:].rearrange("p t (h c) -> p (t h) c", c=C),
                        axis=mybir.AxisListType.X, op=ALU.add)
                    pp = emath.tile([P, TW * H], f32, tag="pp")
                    nc.scalar.activation(out=pp[:], in_=logits[:], func=AF.Exp)

                    wt = wtp.tile([P, TW, OC], f16, tag="wt")
                    nc.vector.tensor_copy(
                        out=wt[:, :, D:OC],
                        in_=pp[:].rearrange("p (t h) -> p t h", h=H))
                    nc.vector.tensor_tensor(
                        out=wt[:, :, 0:D].rearrange("p t (h c) -> p t h c",
                                                    c=C),
                        in0=xl_g[:, :, 0:D].rearrange("p t (h c) -> p t h c",
                                                       c=C),
                        in1=wt[:, :, D:OC].unsqueeze(3).to_broadcast(
                            [P, TW, H, C]),
                        op=ALU.mult)

                    agg = psum.tile([OC, P], f32, tag="agg")
                    for t in range(TW):
                        nc.tensor.matmul(out=agg[:], lhsT=wt[:, t, :],
                                         rhs=oh[:, t, :], start=(t == 0),
                                         stop=(t == TW - 1))

                    if layer == 1:
                        # epilogue: h^T then layer-2 dense for these nodes
                        den = epi.tile([H, P], f32, tag="den")
                        nc.scalar.activation(out=den[:], in_=agg[D:OC, :],
                                             func=AF.Identity, bias=eps4[:])
                        rec = epi.tile([H, P], f32, tag="rec")
                        nc.vector.reciprocal(out=rec[:], in_=den[:])
                        recx = psd.tile([D1, P], f32, tag="recx")
                        nc.tensor.matmul(out=recx[:], lhsT=e4_sb[:],
                                         rhs=rec[:], start=True, stop=True)
                        recs = epi.tile([D1, P], f32, tag="recs")
                        nc.vector.tensor_copy(out=recs[:], in_=recx[:])
                        hpre = epi.tile([D1, P], f32, tag="hpre")
                        nc.vector.tensor_tensor(out=hpre[:], in0=agg[0:D1, :],
                                                in1=recs[:], op=ALU.mult)
                        hb = epi.tile([D1, P], f32, tag="hb")
                        nc.vector.scalar_tensor_tensor(
                            out=hb[:], in0=hpre[:], scalar=1.0,
                            in1=bias1_sb[:, 0:1].to_broadcast([D1, P]),
                            op0=ALU.mult, op1=ALU.add)
                        hT = epi.tile([D1, P], f16, tag="hT")
                        nc.vector.scalar_tensor_tensor(
                            out=hT[:], in0=hb[:], scalar=NEG_ACT, in1=hb[:],
                            op0=ALU.mult, op1=ALU.max)
                        t2 = psd.tile([P, 2 * D2], f32, tag="t2")
                        nc.tensor.matmul(out=t2[:], lhsT=hT[:], rhs=w2c_sb[:],
                                         start=True, stop=True)
                        st2 = epi.tile([P, 2, GW], f16, tag="st2")
                        nc.vector.memset(st2[:], 0)
                        nc.vector.scalar_tensor_tensor(
                            out=st2[:, 0, 0:D2], in0=t2[:, 0:D2], scalar=1.0,
                            in1=b2rep[:, 0:D2], op0=ALU.mult, op1=ALU.add)
                        nc.vector.scalar_tensor_tensor(
                            out=st2[:, 1, 0:D2], in0=t2[:, D2:2 * D2],
                            scalar=1.0, in1=b2rep[:, D2:2 * D2],
                            op0=ALU.mult, op1=ALU.add)
                        nc.sync.dma_start(
                            out=t2l.ap()[w * P:(w + 1) * P, :],
                            in_=st2[:, 0, :])
                        nc.sync.dma_start(
                            out=t2r.ap()[w * P:(w + 1) * P, :],
                            in_=st2[:, 1, :])
                    else:
                        cp = epi.tile([OC2, P], f32, tag="cp")
                        nc.scalar.copy(out=cp[:], in_=agg[:])
                        nc.sync.dma_start(
                            out=out_raw.ap()[:, w * P:(w + 1) * P],
                            in_=cp[:])

            # ---------------- phase 2: layer-1 edges
            edge_layer(1)
            tc.strict_bb_all_engine_barrier()

            # ---------------- phase 3: allgather layer-2 tables
            nc.gpsimd.collective_compute(
                "AllGather", mybir.AluOpType.bypass,
                replica_groups=[list(range(cfg.ncores))],
                ins=[t2l.ap()], outs=[t2lg.ap()])
            nc.gpsimd.collective_compute(
                "AllGather", mybir.AluOpType.bypass,
                replica_groups=[list(range(cfg.ncores))],
                ins=[t2r.ap()], outs=[t2rg.ap()])
            tc.strict_bb_all_engine_barrier()

            # ---------------- phase 4: layer-2 edges
            edge_layer(2)

    nc.compile()
    return nc


# ---------------------------------------------------------------- host prep
def host_prep(x, edge_index, W1l, b1l, W1r, b1r, att1, bias1,
              W2l, b2l, W2r, b2r, att2, bias2, cfg: Cfg):
    """Numpy-only preprocessing: edge sort/pad + weight layouts."""
    n, e, nsh = cfg.n, cfg.e, cfg.nsh
    src = np.asarray(edge_index[0], dtype=np.int64)
    dst = np.asarray(edge_index[1], dtype=np.int64)

    order = np.argsort(dst, kind="stable")
    src_s, dst_s = src[order], dst[order]
    bounds = np.searchsorted(dst_s, np.arange(cfg.ncores + 1) * nsh)

    per_core = []
    nslots = cfg.tt * P
    for k in range(cfg.ncores):
        sl = slice(bounds[k], bounds[k + 1])
        sk, dk = src_s[sl], dst_s[sl]
        dloc = dk - k * nsh
        win = dloc >> 7
        wb = np.searchsorted(dloc, np.arange(cfg.nw + 1) * WIN)
        cnt = np.diff(wb)
        if cnt.max(initial=0) > cfg.tw * P:
            return None  # static schedule overflow -> caller falls back
        pos = (win * (cfg.tw * P) + np.arange(len(dloc)) - wb[win]).astype(
            np.int64)
        sA = np.zeros(nslots, np.int32)
        dA = np.zeros(nslots, np.int32)
        off = np.full(nslots, -1.0, np.float32)
        sA[pos] = sk
        dA[pos] = dk
        off[pos] = (dloc - win * WIN).astype(np.float32)
        sB = (sA // nsh) * cfg.rows2sh + sA % nsh
        dB = np.zeros(nslots, np.int32)
        dB[pos] = k * cfg.rows2sh + dloc

        def wrap(a):
            return np.ascontiguousarray(a.reshape(cfg.tt, P).T)

        per_core.append(dict(srcA=wrap(sA), dstA=wrap(dA),
                             srcB=wrap(sB.astype(np.int32)), dstB=wrap(dB),
                             dstoff=wrap(off)))

    xT = np.zeros((IN + 1, cfg.rows1), np.float16)
    xT[:IN, :n] = np.asarray(x, np.float32).T.astype(np.float16)
    xT[IN, :] = 1.0
    w1c = np.concatenate([np.asarray(W1l), np.asarray(W1r)], axis=0)  # [128,9]
    w1cb = np.concatenate([np.asarray(b1l), np.asarray(b1r)])[None, :]
    w1c_h = np.concatenate([w1c.T, w1cb], axis=0).astype(np.float16)  # [10,128]
    w2c = np.concatenate([np.asarray(W2l), np.asarray(W2r)], axis=0)  # [72,64]
    w2c_h = np.ascontiguousarray(w2c.T).astype(np.float16)  # [64, 72]
    b2c_h = np.concatenate([np.asarray(b2l), np.asarray(b2r)])[None, :].astype(
        np.float32)
    att1_h = np.asarray(att1, np.float32).reshape(1, D1).astype(np.float16)
    att2_h = np.asarray(att2, np.float32).reshape(1, D2).astype(np.float16)
    bias1_h = np.asarray(bias1, np.float32).reshape(D1, 1)
    e4_h = np.zeros((H1, D1), np.float32)
    for h in range(H1):
        e4_h[h, h * HID:(h + 1) * HID] = 1.0

    shared = dict(xT=xT, w1c=w1c_h, w2c=w2c_h, b2c=b2c_h, att1=att1_h,
                  att2=att2_h, bias1=bias1_h, e4=e4_h)
    in_maps = [dict(shared, **pc) for pc in per_core]
    return in_maps


def assemble_output(results, bias2, cfg: Cfg):
    outs = []
    b2 = np.asarray(bias2, np.float32)
    for k in range(cfg.ncores):
        arr = results[k]["out_raw"][:, :cfg.nsh]  # [40, nsh]
        num = arr[:D2].reshape(H2, OUT, cfg.nsh)
        den = arr[D2:D2 + H2]
        outk = (num / (den[:, None, :] + 1e-16)).mean(axis=0).T + b2[None, :]
        outs.append(outk.astype(np.float32))
    return np.concatenate(outs, axis=0)


# ---------------------------------------------------------------- fallback
def _reference_numpy(x, edge_index, W1l, b1l, W1r, b1r, att1, bias1,
                     W2l, b2l, W2r, b2r, att2, bias2):
    """Pure-numpy fallback (used only if inputs don't fit the static plan)."""
    def gatv2(x, src, dst, Wl, bl, Wr, br, att, bias, concat):
        n = x.shape[0]
        H, C = att.shape
        xl = (x @ Wl.T + bl).reshape(n, H, C)
        xr = (x @ Wr.T + br).reshape(n, H, C)
        ee = xl[src] + xr[dst]
        ee = np.where(ee > 0, ee, NEG_ATT * ee)
        logits = np.einsum("ehc,hc->eh", ee, att)
        m = np.full((n, H), -np.inf, np.float32)
        np.maximum.at(m, dst, logits)
        m = np.where(np.isfinite(m), m, 0.0)
        p = np.exp(logits - m[dst])
        den = np.zeros((n, H), np.float32)
        np.add.at(den, dst, p)
        alpha = p / (den[dst] + 1e-16)
        out = np.zeros((n, H, C), np.float32)
        np.add.at(out, dst, alpha[..., None] * xl[src])
        if concat:
            return out.reshape(n, H * C) + bias
        return out.mean(axis=1) + bias

    src, dst = edge_index[0].astype(np.int64), edge_index[1].astype(np.int64)
    h = gatv2(np.asarray(x, np.float32), src, dst, W1l, b1l, W1r, b1r, att1,
              bias1, True)
    h = np.where(h > 0, h, NEG_ACT * h)
    return gatv2(h, src, dst, W2l, b2l, W2r, b2r, att2, bias2, False)


# ---------------------------------------------------------------- entry point
@functools.lru_cache(maxsize=1)
def _compiled():
    return build_program(CFG)


_LAST_RESULTS = {}


def kernel(x, edge_index, W1l, b1l, W1r, b1r, att1, bias1,
           W2l, b2l, W2r, b2r, att2, bias2):
    args = (x, edge_index, W1l, b1l, W1r, b1r, att1, bias1,
            W2l, b2l, W2r, b2r, att2, bias2)
    if (np.asarray(x).shape != (N, IN)
            or np.asarray(edge_index).shape != (2, E)):
        return _reference_numpy(*[np.asarray(a, np.float32) if i != 1 else
                                  np.asarray(a) for i, a in enumerate(args)])

    in_maps = host_prep(*args, CFG)
    if in_maps is None:
        return _reference_numpy(*[np.asarray(a, np.float32) if i != 1 else
                                  np.asarray(a) for i, a in enumerate(args)])

    from concourse.bass_utils import run_bass_kernel_spmd
    nc = _compiled()
    res = run_bass_kernel_spmd(nc, in_maps, core_ids=list(range(NCORES)),
                               trace=False)
    _LAST_RESULTS["res"] = res
    return assemble_output(res.results, bias2, CFG)



# revision 10
# speedup vs baseline: 1.0360x; 1.0360x over previous
"""GATv2 (2-layer, PyG semantics) on 8 Trainium2 NeuronCores.

Strategy (graph/data parallel, dst-sharded):
  - Nodes sharded by destination range across 8 cores (12500 nodes/core).
  - Node tables have 256-B rows [xl_n | xr_n]; per-edge endpoint features
    are fetched with bulk dma_gather (one SWDGE call per shard per group
    of 4 windows, ~0.34ns/descriptor) instead of per-tile indirect DMAs
    (~1us fixed cost each). int16 gather indices are made to fit via a
    signed +/-32K base trick (two shard calls cover 100352 rows).
  - Dst-side rows are gathered from core-LOCAL tables (t1d/t2) with
    group-relative indices so the SPMD program stays core-independent.
  - Edge softmax math on DVE/ACT per region; aggregation via one-hot
    matmuls (stationary one-hot, streamed weights -> node-major agg).
  - Window epilogue is node-major; h is transposed once per window on PE
    to feed the layer-2 dense matmul; raw [num|den] go back to the host
    which finishes divide/mean/bias.
"""

import functools
import sys

import numpy as np

sys.path.insert(0, "/opt/trn_rl_repo")

# ---------------------------------------------------------------- constants
N = 100_000
E = 1_600_000
IN = 9
HID = 16
H1 = 4
H2 = 4
OUT = 9
D1 = H1 * HID  # 64
D2 = H2 * OUT  # 36
NEG_ATT = 0.2
NEG_ACT = 0.01
NCORES = 8
NSH = N // NCORES  # 12500 nodes per core
WIN = 128  # dst nodes per window
P = 128
GW1 = 64  # gather elems layer 1 (fp16) -> 128B
GW2 = 40  # gather elems layer 2 (fp16) -> 80B
RW = 128  # table row width (fp16) -> 256B stride
B0C = 1536  # slots per window, shard block 0 (12 tiles)
B1C = 896  # slots per window, shard block 1 (7 tiles)
B0T = B0C // P
B1T = B1C // P
TWS = B0T + B1T  # 19 tiles per window
SPW = B0C + B1C  # 2432 slots per window
G = 4  # windows per gather group
SHARD = 65536  # shard-0 row threshold


class Cfg:
    def __init__(self, n=N, e=E, ncores=NCORES, dense_chunks=28):
        self.n = n
        self.e = e
        self.ncores = ncores
        self.nsh = n // ncores
        self.nw = -(-self.nsh // WIN)  # 98 windows per core
        self.tt = self.nw * TWS
        self.nslots = self.tt * P
        self.rows1 = ((n + 1023) // 1024) * 1024  # 100352
        self.rowsd = self.nw * WIN  # 12544 local rows
        self.rows2 = self.rowsd * ncores  # 100352
        self.dense_tiles = self.rows1 // P
        self.dense_chunks = dense_chunks
        assert self.dense_tiles % dense_chunks == 0
        self.chunk_tiles = self.dense_tiles // dense_chunks
        self.wb = 7 if self.chunk_tiles % 7 == 0 else 1
        assert self.chunk_tiles % self.wb == 0
        # groups of windows sharing gather calls
        self.groups = []
        w = 0
        while w < self.nw:
            g = min(G, self.nw - w)
            self.groups.append((w, g))
            w += g
        # idx columns per group: (B0C + B1C + SPW) * nwg / 16
        self.gcols = [(B0C + B1C + SPW) * g // 16 for _, g in self.groups]
        self.gcol_off = np.concatenate([[0], np.cumsum(self.gcols)])
        self.idxcols = int(self.gcol_off[-1])
        self.gtile_off = np.concatenate(
            [[0], np.cumsum([TWS * g for _, g in self.groups])])


CFG = Cfg()


# ---------------------------------------------------------------- device code
def build_program(cfg: Cfg):
    """Build the SPMD single-core Bass program (same NEFF on all cores)."""
    import concourse.bacc as bacc
    import concourse.tile as tile
    from concourse import mybir
    from concourse.ap_utils import ap_is_contiguous

    f32 = mybir.dt.float32
    f16 = mybir.dt.float16
    i16 = mybir.dt.int16
    AF = mybir.ActivationFunctionType
    ALU = mybir.AluOpType

    nc = bacc.Bacc("TRN2", target_bir_lowering=False, debug=False,
                   num_devices=cfg.ncores)

    def dma_gather_raw(out_ap, in_ap, idxs_ap, num_idxs, elem_size,
                       elem_step=RW):
        """dma_gather minus the %256 elem-size assert (ucode only needs the
        row STRIDE to be a multiple of 256B; validated on HW)."""
        g = nc.gpsimd
        assert in_ap.ap[0][0] == elem_step
        stride_256, rem = divmod(elem_step * 2, 256)
        assert rem == 0 and stride_256 < 256
        assert ap_is_contiguous(out_ap.ap[1:])
        assert ap_is_contiguous(idxs_ap.ap[1:])
        assert idxs_ap.dtype == i16
        _in_ap = g.lower_ap_dma(in_ap, for_custom_bir_dma=True)
        _idxs_ap = g.lower_ap(idxs_ap)
        _out_ap = g.lower_ap(out_ap)
        return g.add_instruction(
            mybir.InstDMAGatherAnt(
                name=nc.get_next_instruction_name(),
                ins=[*_in_ap, _idxs_ap, g.lower_val_access(g.to_reg(num_idxs))],
                outs=[_out_ap],
                transpose=False, num_idxs=num_idxs, elem_size=elem_size,
                stride_bytes_256=stride_256, gen_mode=0, single_packet=False,
                queue_num=0, sbuf_tokens_per_rank=0, sbuf_free_dim_per_rank=0,
                sbuf_free_dim_pad_per_rank=0, sbuf_byte_offset=0))

    # ---------------- dram I/O
    xT = nc.dram_tensor("xT", [IN + 1, cfg.rows1], f16, kind="ExternalInput")
    xTd = nc.dram_tensor("xTd", [IN + 1, cfg.rowsd], f16,
                         kind="ExternalInput")
    w1c = nc.dram_tensor("w1c", [IN + 1, 2 * D1], f16, kind="ExternalInput")
    w2c = nc.dram_tensor("w2c", [D1, 2 * D2], f16, kind="ExternalInput")
    b2c = nc.dram_tensor("b2c", [1, 2 * D2], f32, kind="ExternalInput")
    att1 = nc.dram_tensor("att1", [1, D1], f16, kind="ExternalInput")
    att2 = nc.dram_tensor("att2", [1, D2], f16, kind="ExternalInput")
    b1row = nc.dram_tensor("b1row", [1, D1], f32, kind="ExternalInput")
    idxA = nc.dram_tensor("idxA", [P, cfg.idxcols], i16, kind="ExternalInput")
    idxB = nc.dram_tensor("idxB", [P, cfg.idxcols], i16, kind="ExternalInput")
    doffA = nc.dram_tensor("doffA", [P, cfg.tt], f32, kind="ExternalInput")
    doffB = nc.dram_tensor("doffB", [P, cfg.tt], f32, kind="ExternalInput")
    out_raw = nc.dram_tensor("out_raw", [cfg.rowsd, D2 + H2], f32,
                             kind="ExternalOutput")

    t1 = nc.dram_tensor("t1", [cfg.rows1, RW], f16)
    t1d = nc.dram_tensor("t1d", [cfg.rowsd, RW], f16)
    t2 = nc.dram_tensor("t2", [cfg.rowsd, RW], f16)
    t2g = nc.dram_tensor("t2g", [cfg.rows2, RW], f16, addr_space="Shared")

    OC1 = D1 + H1  # 68
    OC2 = D2 + H2  # 40
    MAXT0 = G * B0T  # 48 tiles, B0 region of a full group
    MAXT1 = G * B1T  # 28

    with tile.TileContext(nc) as tc:
        import contextlib
        ctx = contextlib.ExitStack()
        with ctx:
            consts = ctx.enter_context(tc.tile_pool(name="consts", bufs=1))
            dofp = ctx.enter_context(tc.tile_pool(name="dofp", bufs=1))
            idxp = ctx.enter_context(tc.tile_pool(name="idxp", bufs=2))
            xtp = ctx.enter_context(tc.tile_pool(name="xtp", bufs=2))
            stage = ctx.enter_context(tc.tile_pool(name="stage", bufs=3))
            gath = ctx.enter_context(tc.tile_pool(name="gath", bufs=2))
            emath = ctx.enter_context(tc.tile_pool(name="emath", bufs=2))
            ohp = ctx.enter_context(tc.tile_pool(name="ohp", bufs=2))
            wtp = ctx.enter_context(tc.tile_pool(name="wtp", bufs=2))
            epi = ctx.enter_context(tc.tile_pool(name="epi", bufs=2))
            psa = ctx.enter_context(tc.tile_pool(name="psa", bufs=1,
                                                 space="PSUM"))
            psd = ctx.enter_context(tc.tile_pool(name="psd", bufs=2,
                                                 space="PSUM"))
            pse = ctx.enter_context(tc.tile_pool(name="pse", bufs=1,
                                                 space="PSUM"))

            # ---------------- constants
            w1c_sb = consts.tile([IN + 1, 2 * D1], f16)
            nc.sync.dma_start(out=w1c_sb[:], in_=w1c.ap())
            w2c_sb = consts.tile([D1, 2 * D2], f16)
            nc.sync.dma_start(out=w2c_sb[:], in_=w2c.ap())
            att1_sb = consts.tile([P, D1], f16)
            nc.sync.dma_start(out=att1_sb[0:1, :], in_=att1.ap())
            nc.gpsimd.partition_broadcast(att1_sb[:], att1_sb[0:1, :])
            att2_sb = consts.tile([P, D2], f16)
            nc.sync.dma_start(out=att2_sb[0:1, :], in_=att2.ap())
            nc.gpsimd.partition_broadcast(att2_sb[:], att2_sb[0:1, :])
            b2rep = consts.tile([P, 2 * D2], f32)
            nc.sync.dma_start(out=b2rep[0:1, :], in_=b2c.ap())
            nc.gpsimd.partition_broadcast(b2rep[:], b2rep[0:1, :])
            b1rep = consts.tile([P, D1], f32)
            nc.sync.dma_start(out=b1rep[0:1, :], in_=b1row.ap())
            nc.gpsimd.partition_broadcast(b1rep[:], b1rep[0:1, :])
            iota_i = consts.tile([P, P], mybir.dt.int32)
            nc.gpsimd.iota(iota_i[:], pattern=[[1, P]], base=0,
                           channel_multiplier=0)
            iota_f = consts.tile([P, P], f32)
            nc.vector.tensor_copy(out=iota_f[:], in_=iota_i[:])
            iota_p = consts.tile([P, P], mybir.dt.int32)
            nc.gpsimd.iota(iota_p[:], pattern=[[0, P]], base=0,
                           channel_multiplier=1)
            iota_pf = consts.tile([P, P], f32)
            nc.vector.tensor_copy(out=iota_pf[:], in_=iota_p[:])
            ident = consts.tile([P, P], f16)
            nc.vector.tensor_tensor(out=ident[:], in0=iota_f[:],
                                    in1=iota_pf[:], op=ALU.is_equal)

            doffA_sb = dofp.tile([P, cfg.tt], f32)
            nc.sync.dma_start(out=doffA_sb[:], in_=doffA.ap())
            doffB_sb = dofp.tile([P, cfg.tt], f32)
            nc.sync.dma_start(out=doffB_sb[:], in_=doffB.ap())

            # ---------------- phase 1: dense node tables (t1 global, t1d loc)
            def dense_pass(src_dram, dst_dram, ntiles, nchunks):
                ck = ntiles // nchunks
                wb = 7 if ck % 7 == 0 else 1
                tv = dst_dram.ap().rearrange("(b t n) f -> b n t f",
                                             t=wb, n=P)
                for c in range(nchunks):
                    xt_sb = xtp.tile([IN + 1, cfg.chunk_tiles * P], f16,
                                     tag="xt")
                    nc.sync.dma_start(
                        out=xt_sb[:, 0:ck * P],
                        in_=src_dram.ap()[:, c * ck * P:(c + 1) * ck * P])
                    for b in range(ck // wb):
                        st = stage.tile([P, wb, 2 * D1], f16, tag="st")
                        for j in range(wb):
                            t = b * wb + j
                            mm = psd.tile([P, 2 * D1], f32, tag="mm")
                            nc.tensor.matmul(out=mm[:],
                                             lhsT=xt_sb[:, t * P:(t + 1) * P],
                                             rhs=w1c_sb[:], start=True,
                                             stop=True)
                            if j % 2 == 0:
                                nc.scalar.copy(out=st[:, j, :], in_=mm[:])
                            else:
                                nc.vector.tensor_copy(out=st[:, j, :],
                                                      in_=mm[:])
                        nc.sync.dma_start(out=tv[c * (ck // wb) + b],
                                          in_=st[:])

            dense_pass(xT, t1, cfg.dense_tiles, cfg.dense_chunks)
            dense_pass(xTd, t1d, cfg.rowsd // P, 7)

            tc.strict_bb_all_engine_barrier()

            # ---------------- edge phase builder
            def edge_layer(layer):
                if layer == 1:
                    D, H, C, OC, GW = D1, H1, HID, OC1, GW1
                    tsrc, tdst = t1, t1d
                    idxL, doff_sb, att_sb = idxA, doffA_sb, att1_sb
                else:
                    D, H, C, OC, GW = D2, H2, OUT, OC2, GW2
                    tsrc, tdst = t2g, t2
                    idxL, doff_sb, att_sb = idxB, doffB_sb, att2_sb

                for gi, (w0, nwg) in enumerate(cfg.groups):
                    t0 = nwg * B0T  # B0 tiles in this group
                    t1n = nwg * B1T
                    tg = t0 + t1n
                    c0 = nwg * B0C // 16  # idx cols per call
                    c1 = nwg * B1C // 16
                    cd = nwg * SPW // 16
                    cbase = int(cfg.gcol_off[gi])
                    tbase = int(cfg.gtile_off[gi])

                    idx_sb = idxp.tile([P, (B0C + B1C + SPW) * G // 16], i16,
                                       tag="idx")
                    nc.sync.dma_start(
                        out=idx_sb[:, 0:c0 + c1 + cd],
                        in_=idxL.ap()[:, cbase:cbase + c0 + c1 + cd])

                    xgs = gath.tile([P, MAXT0 + MAXT1, GW], f16, tag="xgs")
                    xgd = gath.tile([P, MAXT0 + MAXT1, GW], f16, tag="xgd")
                    dma_gather_raw(xgs[:, 0:t0, :],
                                   tsrc.ap()[32768:32769, 0:GW],
                                   idx_sb[:, 0:c0], nwg * B0C, GW)
                    dma_gather_raw(xgs[:, t0:tg, :],
                                   tsrc.ap()[98304:98305, 0:GW],
                                   idx_sb[:, c0:c0 + c1], nwg * B1C, GW)
                    dma_gather_raw(xgd[:, 0:tg, :],
                                   tdst.ap()[w0 * WIN:w0 * WIN + 1, 64:64 + GW],
                                   idx_sb[:, c0 + c1:c0 + c1 + cd],
                                   nwg * SPW, GW)

                    oh = ohp.tile([P, MAXT0 + MAXT1, P], f16, tag="oh")
                    epre = emath.tile([P, MAXT0 + MAXT1, D], f16, tag="epre")
                    ee = emath.tile([P, MAXT0 + MAXT1, D], f16, tag="ee")
                    tmp = emath.tile([P, MAXT0 + MAXT1, D], f16, tag="tmp")
                    logits = emath.tile([P, (MAXT0 + MAXT1) * H], f32,
                                        tag="logits")
                    pp = emath.tile([P, (MAXT0 + MAXT1) * H], f32, tag="pp")
                    wt = wtp.tile([P, MAXT0 + MAXT1, OC], f16, tag="wt")

                    nc.vector.tensor_tensor(
                        out=oh[:, 0:tg, :],
                        in0=iota_f[:].unsqueeze(1).to_broadcast([P, tg, P]),
                        in1=doff_sb[:, tbase:tbase + tg].unsqueeze(
                            2).to_broadcast([P, tg, P]),
                        op=ALU.is_equal)
                    nc.vector.tensor_tensor(out=epre[:, 0:tg, :],
                                            in0=xgs[:, 0:tg, 0:D],
                                            in1=xgd[:, 0:tg, 0:D],
                                            op=ALU.add)
                    nc.vector.scalar_tensor_tensor(
                        out=ee[:, 0:tg, :], in0=epre[:, 0:tg, :],
                        scalar=NEG_ATT, in1=epre[:, 0:tg, :],
                        op0=ALU.mult, op1=ALU.max)
                    nc.vector.tensor_tensor(
                        out=tmp[:, 0:tg, :], in0=ee[:, 0:tg, :],
                        in1=att_sb[:, 0:D].unsqueeze(1).to_broadcast(
                            [P, tg, D]),
                        op=ALU.mult)
                    nc.vector.tensor_reduce(
                        out=logits[:, 0:tg * H],
                        in_=tmp[:, 0:tg, :].rearrange(
                            "p t (h c) -> p (t h) c", c=C),
                        axis=mybir.AxisListType.X, op=ALU.add)
                    nc.scalar.activation(out=pp[:, 0:tg * H],
                                         in_=logits[:, 0:tg * H], func=AF.Exp)
                    nc.vector.tensor_copy(
                        out=wt[:, 0:tg, D:OC],
                        in_=pp[:, 0:tg * H].rearrange("p (t h) -> p t h",
                                                      h=H))
                    nc.vector.tensor_tensor(
                        out=wt[:, 0:tg, 0:D].rearrange(
                            "p t (h c) -> p t h c", c=C),
                        in0=xgs[:, 0:tg, 0:D].rearrange(
                            "p t (h c) -> p t h c", c=C),
                        in1=wt[:, 0:tg, D:OC].unsqueeze(3).to_broadcast(
                            [P, tg, H, C]),
                        op=ALU.mult)

                    for wi in range(nwg):
                        w = w0 + wi
                        # node-major agg: lhsT = one-hot (stationary)
                        agg = psa.tile([P, OC], f32,
                                       tag=f"agg{wi % 2}")
                        tiles = ([wi * B0T + j for j in range(B0T)] +
                                 [t0 + wi * B1T + j for j in range(B1T)])
                        for i, t in enumerate(tiles):
                            nc.tensor.matmul(out=agg[:], lhsT=oh[:, t, :],
                                             rhs=wt[:, t, :], start=(i == 0),
                                             stop=(i == len(tiles) - 1))

                        if layer == 1:
                            den = epi.tile([P, H], f32, tag="den")
                            nc.vector.tensor_scalar(
                                den[:], agg[:, D:OC], 1e-16, None,
                                op0=ALU.add)
                            rec = epi.tile([P, H], f32, tag="rec")
                            nc.vector.reciprocal(out=rec[:], in_=den[:])
                            hpre = epi.tile([P, D1], f32, tag="hpre")
                            nc.vector.tensor_tensor(
                                out=hpre[:].rearrange("p (h c) -> p h c",
                                                      c=HID),
                                in0=agg[:, 0:D1].rearrange(
                                    "p (h c) -> p h c", c=HID),
                                in1=rec[:].unsqueeze(2).to_broadcast(
                                    [P, H1, HID]),
                                op=ALU.mult)
                            hb = epi.tile([P, D1], f32, tag="hb")
                            nc.vector.tensor_tensor(out=hb[:], in0=hpre[:],
                                                    in1=b1rep[:], op=ALU.add)
                            h16 = epi.tile([P, D1], f16, tag="h16")
                            nc.vector.scalar_tensor_tensor(
                                out=h16[:], in0=hb[:], scalar=NEG_ACT,
                                in1=hb[:], op0=ALU.mult, op1=ALU.max)
                            hTp = pse.tile([D1, P], f16, tag="hTp")
                            nc.tensor.transpose(out=hTp[:], in_=h16[:],
                                                identity=ident[:])
                            hT = epi.tile([D1, P], f16, tag="hT")
                            nc.scalar.copy(out=hT[:], in_=hTp[:])
                            t2mm = pse.tile([P, 2 * D2], f32, tag="t2mm")
                            nc.tensor.matmul(out=t2mm[:], lhsT=hT[:],
                                             rhs=w2c_sb[:], start=True,
                                             stop=True)
                            st2 = epi.tile([P, RW], f16, tag="st2")
                            nc.vector.memset(st2[:], 0)
                            nc.vector.scalar_tensor_tensor(
                                out=st2[:, 0:D2], in0=t2mm[:, 0:D2],
                                scalar=1.0, in1=b2rep[:, 0:D2],
                                op0=ALU.mult, op1=ALU.add)
                            nc.vector.scalar_tensor_tensor(
                                out=st2[:, 64:64 + D2],
                                in0=t2mm[:, D2:2 * D2], scalar=1.0,
                                in1=b2rep[:, D2:2 * D2],
                                op0=ALU.mult, op1=ALU.add)
                            nc.sync.dma_start(
                                out=t2.ap()[w * P:(w + 1) * P, :],
                                in_=st2[:])
                        else:
                            cp = epi.tile([P, OC2], f32, tag="cp")
                            nc.scalar.copy(out=cp[:], in_=agg[:])
                            nc.sync.dma_start(
                                out=out_raw.ap()[w * P:(w + 1) * P, :],
                                in_=cp[:])

            # ---------------- phase 2: layer-1 edges
            edge_layer(1)
            tc.strict_bb_all_engine_barrier()

            # ---------------- phase 3: allgather layer-2 table
            nc.gpsimd.collective_compute(
                "AllGather", mybir.AluOpType.bypass,
                replica_groups=[list(range(cfg.ncores))],
                ins=[t2.ap()], outs=[t2g.ap()])
            tc.strict_bb_all_engine_barrier()

            # ---------------- phase 4: layer-2 edges
            edge_layer(2)

    nc.compile()
    return nc


# ---------------------------------------------------------------- host prep
def _wrap16(v):
    """[n] -> [128, n/16]: index j at [j%16, j//16], replicated x8."""
    a = v.reshape(-1, 16).T  # [16, n/16]
    return np.tile(a, (8, 1))


def host_prep(x, edge_index, W1l, b1l, W1r, b1r, att1, bias1,
              W2l, b2l, W2r, b2r, att2, bias2, cfg: Cfg):
    """Numpy-only preprocessing: per-layer slot layouts + weight layouts."""
    n, nsh = cfg.n, cfg.nsh
    src = np.asarray(edge_index[0], dtype=np.int64)
    dst = np.asarray(edge_index[1], dtype=np.int64)

    order = np.argsort(dst, kind="stable")
    src_s, dst_s = src[order], dst[order]
    bounds = np.searchsorted(dst_s, np.arange(cfg.ncores + 1) * nsh)

    ngr = len(cfg.groups)
    grp_sizes = np.array([g for _, g in cfg.groups], np.int64)
    grp_slot_base = np.concatenate([[0], np.cumsum(grp_sizes * SPW)])
    grp_of_w = np.repeat(np.arange(ngr), grp_sizes)
    wi_of_w = np.concatenate([np.arange(g) for g in grp_sizes])
    w0_of_g = np.array([w for w, _ in cfg.groups], np.int64)

    def build_layer(sk, dloc, rowv):
        """Slot layout for one layer. rowv = table row of src per edge."""
        win = dloc >> 7
        blk = (rowv >= SHARD).astype(np.int64)
        key = win * 2 + blk
        ord2 = np.argsort(key, kind="stable")
        ks = key[ord2]
        cnt = np.bincount(ks, minlength=cfg.nw * 2)
        if (cnt[0::2] > B0C - 1).any() or (cnt[1::2] > B1C - 1).any():
            return None
        starts = np.concatenate([[0], np.cumsum(cnt)])[:-1]
        within = np.arange(len(ks)) - starts[ks]
        wv, bv = ks >> 1, ks & 1
        gv = grp_of_w[wv]
        wiv = wi_of_w[wv]
        nwgv = grp_sizes[gv]
        base = grp_slot_base[gv] + np.where(
            bv == 0, wiv * B0C, nwgv * B0C + wiv * B1C)
        slot = np.empty(len(ks), np.int64)
        slot[ord2] = base + within

        src16 = np.zeros(cfg.nslots, np.int16)
        src16[slot] = (rowv - np.where(blk == 1, 98304, 32768)).astype(
            np.int16)
        dst16 = np.zeros(cfg.nslots, np.int16)
        dst16[slot] = (dloc - w0_of_g[grp_of_w[win]] * WIN).astype(np.int16)
        dof = np.full(cfg.nslots, -1.0, np.float32)
        dof[slot] = (dloc & 127).astype(np.float32)

        cols = []
        for gi, (w0g, nwg) in enumerate(cfg.groups):
            s0 = int(grp_slot_base[gi])
            b0n, b1n = nwg * B0C, nwg * B1C
            cols.append(_wrap16(src16[s0:s0 + b0n]))
            cols.append(_wrap16(src16[s0 + b0n:s0 + b0n + b1n]))
            cols.append(_wrap16(dst16[s0:s0 + b0n + b1n]))
        idx = np.ascontiguousarray(np.concatenate(cols, axis=1))
        dofw = np.ascontiguousarray(dof.reshape(cfg.tt, P).T)
        return idx, dofw

    r2 = (src // nsh) * cfg.rowsd + src % nsh  # layer-2 table row per node id
    r2_s = r2[order]

    per_core = []
    for k in range(cfg.ncores):
        sl = slice(bounds[k], bounds[k + 1])
        sk, dk = src_s[sl], dst_s[sl]
        dloc = dk - k * nsh
        a = build_layer(sk, dloc, sk)
        b = build_layer(sk, dloc, r2_s[sl])
        if a is None or b is None:
            return None
        xTd = np.zeros((IN + 1, cfg.rowsd), np.float16)
        xTd[:IN, :nsh] = np.asarray(
            x[k * nsh:(k + 1) * nsh], np.float32).T.astype(np.float16)
        xTd[IN, :] = 1.0
        per_core.append(dict(idxA=a[0], doffA=a[1], idxB=b[0], doffB=b[1],
                             xTd=xTd))

    xT = np.zeros((IN + 1, cfg.rows1), np.float16)
    xT[:IN, :n] = np.asarray(x, np.float32).T.astype(np.float16)
    xT[IN, :] = 1.0
    w1c = np.concatenate([np.asarray(W1l), np.asarray(W1r)], axis=0)
    w1cb = np.concatenate([np.asarray(b1l), np.asarray(b1r)])[None, :]
    w1c_h = np.concatenate([w1c.T, w1cb], axis=0).astype(np.float16)
    w2c = np.concatenate([np.asarray(W2l), np.asarray(W2r)], axis=0)
    w2c_h = np.ascontiguousarray(w2c.T).astype(np.float16)
    b2c_h = np.concatenate([np.asarray(b2l), np.asarray(b2r)])[None, :].astype(
        np.float32)
    att1_h = np.asarray(att1, np.float32).reshape(1, D1).astype(np.float16)
    att2_h = np.asarray(att2, np.float32).reshape(1, D2).astype(np.float16)
    b1row_h = np.asarray(bias1, np.float32).reshape(1, D1)

    shared = dict(xT=xT, w1c=w1c_h, w2c=w2c_h, b2c=b2c_h, att1=att1_h,
                  att2=att2_h, b1row=b1row_h)
    return [dict(shared, **pc) for pc in per_core]


def assemble_output(results, bias2, cfg: Cfg):
    outs = []
    b2 = np.asarray(bias2, np.float32)
    for k in range(cfg.ncores):
        arr = results[k]["out_raw"][:cfg.nsh]  # [nsh, 40] node-major
        num = arr[:, :D2].reshape(cfg.nsh, H2, OUT)
        den = arr[:, D2:D2 + H2]
        outk = (num / (den[:, :, None] + 1e-16)).mean(axis=1) + b2[None, :]
        outs.append(outk.astype(np.float32))
    return np.concatenate(outs, axis=0)


# ---------------------------------------------------------------- fallback
def _reference_numpy(x, edge_index, W1l, b1l, W1r, b1r, att1, bias1,
                     W2l, b2l, W2r, b2r, att2, bias2):
    def gatv2(x, src, dst, Wl, bl, Wr, br, att, bias, concat):
        n = x.shape[0]
        H, C = att.shape
        xl = (x @ Wl.T + bl).reshape(n, H, C)
        xr = (x @ Wr.T + br).reshape(n, H, C)
        ee = xl[src] + xr[dst]
        ee = np.where(ee > 0, ee, NEG_ATT * ee)
        logits = np.einsum("ehc,hc->eh", ee, att)
        m = np.full((n, H), -np.inf, np.float32)
        np.maximum.at(m, dst, logits)
        m = np.where(np.isfinite(m), m, 0.0)
        p = np.exp(logits - m[dst])
        den = np.zeros((n, H), np.float32)
        np.add.at(den, dst, p)
        alpha = p / (den[dst] + 1e-16)
        out = np.zeros((n, H, C), np.float32)
        np.add.at(out, dst, alpha[..., None] * xl[src])
        if concat:
            return out.reshape(n, H * C) + bias
        return out.mean(axis=1) + bias

    src, dst = edge_index[0].astype(np.int64), edge_index[1].astype(np.int64)
    h = gatv2(np.asarray(x, np.float32), src, dst, W1l, b1l, W1r, b1r, att1,
              bias1, True)
    h = np.where(h > 0, h, NEG_ACT * h)
    return gatv2(h, src, dst, W2l, b2l, W2r, b2r, att2, bias2, False)


# ---------------------------------------------------------------- entry point
@functools.lru_cache(maxsize=1)
def _compiled():
    return build_program(CFG)


_LAST_RESULTS = {}


def kernel(x, edge_index, W1l, b1l, W1r, b1r, att1, bias1,
           W2l, b2l, W2r, b2r, att2, bias2):
    args = (x, edge_index, W1l, b1l, W1r, b1r, att1, bias1,
            W2l, b2l, W2r, b2r, att2, bias2)
    if (np.asarray(x).shape != (N, IN)
            or np.asarray(edge_index).shape != (2, E)):
        return _reference_numpy(*[np.asarray(a, np.float32) if i != 1 else
                                  np.asarray(a) for i, a in enumerate(args)])

    in_maps = host_prep(*args, CFG)
    if in_maps is None:
        return _reference_numpy(*[np.asarray(a, np.float32) if i != 1 else
                                  np.asarray(a) for i, a in enumerate(args)])

    from concourse.bass_utils import run_bass_kernel_spmd
    nc = _compiled()
    res = run_bass_kernel_spmd(nc, in_maps, core_ids=list(range(NCORES)),
                               trace=False)
    _LAST_RESULTS["res"] = res
    return assemble_output(res.results, bias2, CFG)


# revision 13
# speedup vs baseline: 2.2697x; 2.1907x over previous
"""GATv2 (2-layer, PyG semantics) on 8 Trainium2 NeuronCores.

Strategy (graph/data parallel, dst-sharded):
  - Nodes sharded by destination range across 8 cores (12500 nodes/core).
  - Node tables have 256-B rows [xl_n | xr_n]; per-edge endpoint features
    are fetched with bulk dma_gather (one SWDGE call per shard per group
    of 4 windows, ~0.34ns/descriptor) instead of per-tile indirect DMAs
    (~1us fixed cost each). int16 gather indices are made to fit via a
    signed +/-32K base trick (two shard calls cover 100352 rows).
  - Dst-side rows are gathered from core-LOCAL tables (t1d/t2) with
    group-relative indices so the SPMD program stays core-independent.
  - Edge softmax math on DVE/ACT per region; aggregation via one-hot
    matmuls (stationary one-hot, streamed weights -> node-major agg).
  - Window epilogue is node-major; h is transposed once per window on PE
    to feed the layer-2 dense matmul; raw [num|den] go back to the host
    which finishes divide/mean/bias.
"""

import functools
import sys

import numpy as np

sys.path.insert(0, "/opt/trn_rl_repo")

# ---------------------------------------------------------------- constants
N = 100_000
E = 1_600_000
IN = 9
HID = 16
H1 = 4
H2 = 4
OUT = 9
D1 = H1 * HID  # 64
D2 = H2 * OUT  # 36
NEG_ATT = 0.2
NEG_ACT = 0.01
NCORES = 8
NSH = N // NCORES  # 12500 nodes per core
WIN = 128  # dst nodes per window
P = 128
GW1 = 64  # gather elems layer 1 (fp16) -> 128B
GW2 = 40  # gather elems layer 2 (fp16) -> 80B
RW = 128  # table row width (fp16) -> 256B stride
B0C = 1536  # slots per window, shard block 0 (12 tiles)
B1C = 896  # slots per window, shard block 1 (7 tiles)
B0T = B0C // P
B1T = B1C // P
TWS = B0T + B1T  # 19 tiles per window
SPW = B0C + B1C  # 2432 slots per window
G = 4  # windows per gather group
SHARD = 65536  # shard-0 row threshold


class Cfg:
    def __init__(self, n=N, e=E, ncores=NCORES, dense_chunks=28):
        self.n = n
        self.e = e
        self.ncores = ncores
        self.nsh = n // ncores
        self.nw = -(-self.nsh // WIN)  # 98 windows per core
        self.tt = self.nw * TWS
        self.nslots = self.tt * P
        self.rows1 = ((n + 1023) // 1024) * 1024  # 100352
        self.rowsd = self.nw * WIN  # 12544 local rows
        self.rows2 = self.rowsd * ncores  # 100352
        self.dense_tiles = self.rows1 // P
        self.dense_chunks = dense_chunks
        assert self.dense_tiles % dense_chunks == 0
        self.chunk_tiles = self.dense_tiles // dense_chunks
        self.wb = 7 if self.chunk_tiles % 7 == 0 else 1
        assert self.chunk_tiles % self.wb == 0
        # groups of windows sharing gather calls
        self.groups = []
        w = 0
        while w < self.nw:
            g = min(G, self.nw - w)
            self.groups.append((w, g))
            w += g
        # idx columns per group: (B0C + B1C + SPW) * nwg / 16
        self.gcols = [(B0C + B1C + SPW) * g // 16 for _, g in self.groups]
        self.gcol_off = np.concatenate([[0], np.cumsum(self.gcols)])
        self.idxcols = int(self.gcol_off[-1])
        self.gtile_off = np.concatenate(
            [[0], np.cumsum([TWS * g for _, g in self.groups])])


CFG = Cfg()


# ---------------------------------------------------------------- device code
def build_program(cfg: Cfg):
    """Build the SPMD single-core Bass program (same NEFF on all cores)."""
    import concourse.bacc as bacc
    import concourse.tile as tile
    from concourse import mybir
    from concourse.ap_utils import ap_is_contiguous

    f32 = mybir.dt.float32
    f16 = mybir.dt.float16
    i16 = mybir.dt.int16
    AF = mybir.ActivationFunctionType
    ALU = mybir.AluOpType

    nc = bacc.Bacc("TRN2", target_bir_lowering=False, debug=False,
                   num_devices=cfg.ncores, num_swdge_queues=4)

    def dma_gather_raw(out_ap, in_ap, idxs_ap, num_idxs, elem_size,
                       elem_step=RW, queue_num=0):
        """dma_gather minus the %256 elem-size assert (ucode only needs the
        row STRIDE to be a multiple of 256B; validated on HW)."""
        g = nc.gpsimd
        assert in_ap.ap[0][0] == elem_step
        stride_256, rem = divmod(elem_step * 2, 256)
        assert rem == 0 and stride_256 < 256
        assert ap_is_contiguous(out_ap.ap[1:])
        assert ap_is_contiguous(idxs_ap.ap[1:])
        assert idxs_ap.dtype == i16
        _in_ap = g.lower_ap_dma(in_ap, for_custom_bir_dma=True)
        _idxs_ap = g.lower_ap(idxs_ap)
        _out_ap = g.lower_ap(out_ap)
        return g.add_instruction(
            mybir.InstDMAGatherAnt(
                name=nc.get_next_instruction_name(),
                ins=[*_in_ap, _idxs_ap, g.lower_val_access(g.to_reg(num_idxs))],
                outs=[_out_ap],
                transpose=False, num_idxs=num_idxs, elem_size=elem_size,
                stride_bytes_256=stride_256, gen_mode=0, single_packet=False,
                queue_num=queue_num, sbuf_tokens_per_rank=0,
                sbuf_free_dim_per_rank=0,
                sbuf_free_dim_pad_per_rank=0, sbuf_byte_offset=0))

    # ---------------- dram I/O
    xT = nc.dram_tensor("xT", [IN + 1, cfg.rows1], f16, kind="ExternalInput")
    xTd = nc.dram_tensor("xTd", [IN + 1, cfg.rowsd], f16,
                         kind="ExternalInput")
    w1c = nc.dram_tensor("w1c", [IN + 1, 2 * D1], f16, kind="ExternalInput")
    w2c = nc.dram_tensor("w2c", [D1, 2 * D2], f16, kind="ExternalInput")
    b2c = nc.dram_tensor("b2c", [1, 2 * D2], f32, kind="ExternalInput")
    att1 = nc.dram_tensor("att1", [1, D1], f16, kind="ExternalInput")
    att2 = nc.dram_tensor("att2", [1, D2], f16, kind="ExternalInput")
    b1row = nc.dram_tensor("b1row", [1, D1], f32, kind="ExternalInput")
    idxA = nc.dram_tensor("idxA", [P, cfg.idxcols], i16, kind="ExternalInput")
    idxB = nc.dram_tensor("idxB", [P, cfg.idxcols], i16, kind="ExternalInput")
    doffA = nc.dram_tensor("doffA", [P, cfg.tt], f32, kind="ExternalInput")
    doffB = nc.dram_tensor("doffB", [P, cfg.tt], f32, kind="ExternalInput")
    out_raw = nc.dram_tensor("out_raw", [cfg.rowsd, D2 + H2], f32,
                             kind="ExternalOutput")

    t1 = nc.dram_tensor("t1", [cfg.rows1, RW], f16)
    t1d = nc.dram_tensor("t1d", [cfg.rowsd, RW], f16)
    t2 = nc.dram_tensor("t2", [cfg.rowsd, RW], f16)
    t2g = nc.dram_tensor("t2g", [cfg.rows2, RW], f16, addr_space="Shared")

    OC1 = D1 + H1  # 68
    OC2 = D2 + H2  # 40
    MAXT0 = G * B0T  # 48 tiles, B0 region of a full group
    MAXT1 = G * B1T  # 28

    with tile.TileContext(nc) as tc:
        import contextlib
        ctx = contextlib.ExitStack()
        with ctx:
            consts = ctx.enter_context(tc.tile_pool(name="consts", bufs=1))
            dofp = ctx.enter_context(tc.tile_pool(name="dofp", bufs=1))
            idxp = ctx.enter_context(tc.tile_pool(name="idxp", bufs=2))
            xtp = ctx.enter_context(tc.tile_pool(name="xtp", bufs=2))
            stage = ctx.enter_context(tc.tile_pool(name="stage", bufs=3))
            gath = ctx.enter_context(tc.tile_pool(name="gath", bufs=2))
            emath = ctx.enter_context(tc.tile_pool(name="emath", bufs=2))
            ohp = ctx.enter_context(tc.tile_pool(name="ohp", bufs=2))
            wtp = ctx.enter_context(tc.tile_pool(name="wtp", bufs=2))
            epi = ctx.enter_context(tc.tile_pool(name="epi", bufs=2))
            psa = ctx.enter_context(tc.tile_pool(name="psa", bufs=1,
                                                 space="PSUM"))
            psd = ctx.enter_context(tc.tile_pool(name="psd", bufs=2,
                                                 space="PSUM"))
            pse = ctx.enter_context(tc.tile_pool(name="pse", bufs=1,
                                                 space="PSUM"))

            # ---------------- constants
            w1c_sb = consts.tile([IN + 1, 2 * D1], f16)
            nc.sync.dma_start(out=w1c_sb[:], in_=w1c.ap())
            w2c_sb = consts.tile([D1, 2 * D2], f16)
            nc.sync.dma_start(out=w2c_sb[:], in_=w2c.ap())
            att1_sb = consts.tile([P, D1], f16)
            nc.sync.dma_start(out=att1_sb[0:1, :], in_=att1.ap())
            nc.gpsimd.partition_broadcast(att1_sb[:], att1_sb[0:1, :])
            att2_sb = consts.tile([P, D2], f16)
            nc.sync.dma_start(out=att2_sb[0:1, :], in_=att2.ap())
            nc.gpsimd.partition_broadcast(att2_sb[:], att2_sb[0:1, :])
            b2rep = consts.tile([P, 2 * D2], f32)
            nc.sync.dma_start(out=b2rep[0:1, :], in_=b2c.ap())
            nc.gpsimd.partition_broadcast(b2rep[:], b2rep[0:1, :])
            b1rep = consts.tile([P, D1], f32)
            nc.sync.dma_start(out=b1rep[0:1, :], in_=b1row.ap())
            nc.gpsimd.partition_broadcast(b1rep[:], b1rep[0:1, :])
            iota_i = consts.tile([P, P], mybir.dt.int32)
            nc.gpsimd.iota(iota_i[:], pattern=[[1, P]], base=0,
                           channel_multiplier=0)
            iota_f = consts.tile([P, P], f32)
            nc.vector.tensor_copy(out=iota_f[:], in_=iota_i[:])
            iota_p = consts.tile([P, P], mybir.dt.int32)
            nc.gpsimd.iota(iota_p[:], pattern=[[0, P]], base=0,
                           channel_multiplier=1)
            iota_pf = consts.tile([P, P], f32)
            nc.vector.tensor_copy(out=iota_pf[:], in_=iota_p[:])
            ident = consts.tile([P, P], f16)
            nc.vector.tensor_tensor(out=ident[:], in0=iota_f[:],
                                    in1=iota_pf[:], op=ALU.is_equal)

            doffA_sb = dofp.tile([P, cfg.tt], f32)
            nc.sync.dma_start(out=doffA_sb[:], in_=doffA.ap())
            doffB_sb = dofp.tile([P, cfg.tt], f32)
            nc.sync.dma_start(out=doffB_sb[:], in_=doffB.ap())

            # ---------------- phase 1: dense node tables (t1 global, t1d loc)
            def dense_pass(src_dram, dst_dram, ntiles, nchunks):
                ck = ntiles // nchunks
                wb = 7 if ck % 7 == 0 else 1
                tv = dst_dram.ap().rearrange("(b t n) f -> b n t f",
                                             t=wb, n=P)
                for c in range(nchunks):
                    xt_sb = xtp.tile([IN + 1, cfg.chunk_tiles * P], f16,
                                     tag="xt")
                    nc.sync.dma_start(
                        out=xt_sb[:, 0:ck * P],
                        in_=src_dram.ap()[:, c * ck * P:(c + 1) * ck * P])
                    for b in range(ck // wb):
                        st = stage.tile([P, wb, 2 * D1], f16, tag="st")
                        for j in range(wb):
                            t = b * wb + j
                            mm = psd.tile([P, 2 * D1], f32, tag="mm")
                            nc.tensor.matmul(out=mm[:],
                                             lhsT=xt_sb[:, t * P:(t + 1) * P],
                                             rhs=w1c_sb[:], start=True,
                                             stop=True)
                            if j % 2 == 0:
                                nc.scalar.copy(out=st[:, j, :], in_=mm[:])
                            else:
                                nc.vector.tensor_copy(out=st[:, j, :],
                                                      in_=mm[:])
                        nc.sync.dma_start(out=tv[c * (ck // wb) + b],
                                          in_=st[:])

            dense_pass(xT, t1, cfg.dense_tiles, cfg.dense_chunks)
            dense_pass(xTd, t1d, cfg.rowsd // P, 7)

            tc.strict_bb_all_engine_barrier()

            # ---------------- edge phase builder
            def edge_layer(layer):
                if layer == 1:
                    D, H, C, OC, GW = D1, H1, HID, OC1, GW1
                    tsrc, tdst = t1, t1d
                    idxL, doff_sb, att_sb = idxA, doffA_sb, att1_sb
                else:
                    D, H, C, OC, GW = D2, H2, OUT, OC2, GW2
                    tsrc, tdst = t2g, t2
                    idxL, doff_sb, att_sb = idxB, doffB_sb, att2_sb

                for gi, (w0, nwg) in enumerate(cfg.groups):
                    t0 = nwg * B0T  # B0 tiles in this group
                    t1n = nwg * B1T
                    tg = t0 + t1n
                    c0 = nwg * B0C // 16  # idx cols per call
                    c1 = nwg * B1C // 16
                    cd = nwg * SPW // 16
                    cbase = int(cfg.gcol_off[gi])
                    tbase = int(cfg.gtile_off[gi])

                    idx_sb = idxp.tile([P, (B0C + B1C + SPW) * G // 16], i16,
                                       tag="idx")
                    nc.sync.dma_start(
                        out=idx_sb[:, 0:c0 + c1 + cd],
                        in_=idxL.ap()[:, cbase:cbase + c0 + c1 + cd])

                    xgs = gath.tile([P, MAXT0 + MAXT1, GW], f16, tag="xgs")
                    xgd = gath.tile([P, MAXT0 + MAXT1, GW], f16, tag="xgd")
                    dma_gather_raw(xgs[:, 0:t0, :],
                                   tsrc.ap()[32768:32769, 0:GW],
                                   idx_sb[:, 0:c0], nwg * B0C, GW,
                                   queue_num=0)
                    dma_gather_raw(xgs[:, t0:tg, :],
                                   tsrc.ap()[98304:98305, 0:GW],
                                   idx_sb[:, c0:c0 + c1], nwg * B1C, GW,
                                   queue_num=1)
                    th = tg // 2  # split dst gather across queues 2 and 3
                    dma_gather_raw(xgd[:, 0:th, :],
                                   tdst.ap()[w0 * WIN:w0 * WIN + 1, 64:64 + GW],
                                   idx_sb[:, c0 + c1:c0 + c1 + cd // 2],
                                   th * P, GW, queue_num=2)
                    dma_gather_raw(xgd[:, th:tg, :],
                                   tdst.ap()[w0 * WIN:w0 * WIN + 1, 64:64 + GW],
                                   idx_sb[:, c0 + c1 + cd // 2:c0 + c1 + cd],
                                   (tg - th) * P, GW, queue_num=3)

                    oh = ohp.tile([P, MAXT0 + MAXT1, P], f16, tag="oh")
                    epre = emath.tile([P, MAXT0 + MAXT1, D], f16, tag="epre")
                    ee = emath.tile([P, MAXT0 + MAXT1, D], f16, tag="ee")
                    tmp = emath.tile([P, MAXT0 + MAXT1, D], f16, tag="tmp")
                    logits = emath.tile([P, (MAXT0 + MAXT1) * H], f32,
                                        tag="logits")
                    pp = emath.tile([P, (MAXT0 + MAXT1) * H], f32, tag="pp")
                    wt = wtp.tile([P, MAXT0 + MAXT1, OC], f16, tag="wt")

                    nc.vector.tensor_tensor(
                        out=oh[:, 0:tg, :],
                        in0=iota_f[:].unsqueeze(1).to_broadcast([P, tg, P]),
                        in1=doff_sb[:, tbase:tbase + tg].unsqueeze(
                            2).to_broadcast([P, tg, P]),
                        op=ALU.is_equal)
                    nc.vector.tensor_tensor(out=epre[:, 0:tg, :],
                                            in0=xgs[:, 0:tg, 0:D],
                                            in1=xgd[:, 0:tg, 0:D],
                                            op=ALU.add)
                    nc.vector.scalar_tensor_tensor(
                        out=ee[:, 0:tg, :], in0=epre[:, 0:tg, :],
                        scalar=NEG_ATT, in1=epre[:, 0:tg, :],
                        op0=ALU.mult, op1=ALU.max)
                    nc.vector.tensor_tensor(
                        out=tmp[:, 0:tg, :], in0=ee[:, 0:tg, :],
                        in1=att_sb[:, 0:D].unsqueeze(1).to_broadcast(
                            [P, tg, D]),
                        op=ALU.mult)
                    nc.vector.tensor_reduce(
                        out=logits[:, 0:tg * H],
                        in_=tmp[:, 0:tg, :].rearrange(
                            "p t (h c) -> p (t h) c", c=C),
                        axis=mybir.AxisListType.X, op=ALU.add)
                    nc.scalar.activation(out=pp[:, 0:tg * H],
                                         in_=logits[:, 0:tg * H], func=AF.Exp)
                    nc.vector.tensor_copy(
                        out=wt[:, 0:tg, D:OC],
                        in_=pp[:, 0:tg * H].rearrange("p (t h) -> p t h",
                                                      h=H))
                    nc.vector.tensor_tensor(
                        out=wt[:, 0:tg, 0:D].rearrange(
                            "p t (h c) -> p t h c", c=C),
                        in0=xgs[:, 0:tg, 0:D].rearrange(
                            "p t (h c) -> p t h c", c=C),
                        in1=wt[:, 0:tg, D:OC].unsqueeze(3).to_broadcast(
                            [P, tg, H, C]),
                        op=ALU.mult)

                    for wi in range(nwg):
                        w = w0 + wi
                        # node-major agg: lhsT = one-hot (stationary)
                        agg = psa.tile([P, OC], f32,
                                       tag=f"agg{wi % 2}")
                        tiles = ([wi * B0T + j for j in range(B0T)] +
                                 [t0 + wi * B1T + j for j in range(B1T)])
                        for i, t in enumerate(tiles):
                            nc.tensor.matmul(out=agg[:], lhsT=oh[:, t, :],
                                             rhs=wt[:, t, :], start=(i == 0),
                                             stop=(i == len(tiles) - 1))

                        if layer == 1:
                            den = epi.tile([P, H], f32, tag="den")
                            nc.vector.tensor_scalar(
                                den[:], agg[:, D:OC], 1e-16, None,
                                op0=ALU.add)
                            rec = epi.tile([P, H], f32, tag="rec")
                            nc.vector.reciprocal(out=rec[:], in_=den[:])
                            hpre = epi.tile([P, D1], f32, tag="hpre")
                            nc.vector.tensor_tensor(
                                out=hpre[:].rearrange("p (h c) -> p h c",
                                                      c=HID),
                                in0=agg[:, 0:D1].rearrange(
                                    "p (h c) -> p h c", c=HID),
                                in1=rec[:].unsqueeze(2).to_broadcast(
                                    [P, H1, HID]),
                                op=ALU.mult)
                            hb = epi.tile([P, D1], f32, tag="hb")
                            nc.vector.tensor_tensor(out=hb[:], in0=hpre[:],
                                                    in1=b1rep[:], op=ALU.add)
                            h16 = epi.tile([P, D1], f16, tag="h16")
                            nc.vector.scalar_tensor_tensor(
                                out=h16[:], in0=hb[:], scalar=NEG_ACT,
                                in1=hb[:], op0=ALU.mult, op1=ALU.max)
                            hTp = pse.tile([D1, P], f16, tag="hTp")
                            nc.tensor.transpose(out=hTp[:], in_=h16[:],
                                                identity=ident[:])
                            hT = epi.tile([D1, P], f16, tag="hT")
                            nc.scalar.copy(out=hT[:], in_=hTp[:])
                            t2mm = pse.tile([P, 2 * D2], f32, tag="t2mm")
                            nc.tensor.matmul(out=t2mm[:], lhsT=hT[:],
                                             rhs=w2c_sb[:], start=True,
                                             stop=True)
                            st2 = epi.tile([P, RW], f16, tag="st2")
                            nc.vector.memset(st2[:], 0)
                            nc.vector.scalar_tensor_tensor(
                                out=st2[:, 0:D2], in0=t2mm[:, 0:D2],
                                scalar=1.0, in1=b2rep[:, 0:D2],
                                op0=ALU.mult, op1=ALU.add)
                            nc.vector.scalar_tensor_tensor(
                                out=st2[:, 64:64 + D2],
                                in0=t2mm[:, D2:2 * D2], scalar=1.0,
                                in1=b2rep[:, D2:2 * D2],
                                op0=ALU.mult, op1=ALU.add)
                            nc.sync.dma_start(
                                out=t2.ap()[w * P:(w + 1) * P, :],
                                in_=st2[:])
                        else:
                            cp = epi.tile([P, OC2], f32, tag="cp")
                            nc.scalar.copy(out=cp[:], in_=agg[:])
                            nc.sync.dma_start(
                                out=out_raw.ap()[w * P:(w + 1) * P, :],
                                in_=cp[:])

            # ---------------- phase 2: layer-1 edges
            edge_layer(1)
            tc.strict_bb_all_engine_barrier()

            # ---------------- phase 3: allgather layer-2 table
            nc.gpsimd.collective_compute(
                "AllGather", mybir.AluOpType.bypass,
                replica_groups=[list(range(cfg.ncores))],
                ins=[t2.ap()], outs=[t2g.ap()])
            tc.strict_bb_all_engine_barrier()

            # ---------------- phase 4: layer-2 edges
            edge_layer(2)

    nc.compile()
    return nc


# ---------------------------------------------------------------- host prep
def _wrap16(v):
    """[n] -> [128, n/16]: index j at [j%16, j//16], replicated x8."""
    a = v.reshape(-1, 16).T  # [16, n/16]
    return np.tile(a, (8, 1))


def host_prep(x, edge_index, W1l, b1l, W1r, b1r, att1, bias1,
              W2l, b2l, W2r, b2r, att2, bias2, cfg: Cfg):
    """Numpy-only preprocessing: per-layer slot layouts + weight layouts."""
    n, nsh = cfg.n, cfg.nsh
    src = np.asarray(edge_index[0], dtype=np.int64)
    dst = np.asarray(edge_index[1], dtype=np.int64)

    order = np.argsort(dst, kind="stable")
    src_s, dst_s = src[order], dst[order]
    bounds = np.searchsorted(dst_s, np.arange(cfg.ncores + 1) * nsh)

    ngr = len(cfg.groups)
    grp_sizes = np.array([g for _, g in cfg.groups], np.int64)
    grp_slot_base = np.concatenate([[0], np.cumsum(grp_sizes * SPW)])
    grp_of_w = np.repeat(np.arange(ngr), grp_sizes)
    wi_of_w = np.concatenate([np.arange(g) for g in grp_sizes])
    w0_of_g = np.array([w for w, _ in cfg.groups], np.int64)

    def build_layer(sk, dloc, rowv):
        """Slot layout for one layer. rowv = table row of src per edge."""
        win = dloc >> 7
        blk = (rowv >= SHARD).astype(np.int64)
        key = win * 2 + blk
        ord2 = np.argsort(key, kind="stable")
        ks = key[ord2]
        cnt = np.bincount(ks, minlength=cfg.nw * 2)
        if (cnt[0::2] > B0C - 1).any() or (cnt[1::2] > B1C - 1).any():
            return None
        starts = np.concatenate([[0], np.cumsum(cnt)])[:-1]
        within = np.arange(len(ks)) - starts[ks]
        wv, bv = ks >> 1, ks & 1
        gv = grp_of_w[wv]
        wiv = wi_of_w[wv]
        nwgv = grp_sizes[gv]
        base = grp_slot_base[gv] + np.where(
            bv == 0, wiv * B0C, nwgv * B0C + wiv * B1C)
        slot = np.empty(len(ks), np.int64)
        slot[ord2] = base + within

        src16 = np.zeros(cfg.nslots, np.int16)
        src16[slot] = (rowv - np.where(blk == 1, 98304, 32768)).astype(
            np.int16)
        dst16 = np.zeros(cfg.nslots, np.int16)
        dst16[slot] = (dloc - w0_of_g[grp_of_w[win]] * WIN).astype(np.int16)
        dof = np.full(cfg.nslots, -1.0, np.float32)
        dof[slot] = (dloc & 127).astype(np.float32)

        cols = []
        for gi, (w0g, nwg) in enumerate(cfg.groups):
            s0 = int(grp_slot_base[gi])
            b0n, b1n = nwg * B0C, nwg * B1C
            cols.append(_wrap16(src16[s0:s0 + b0n]))
            cols.append(_wrap16(src16[s0 + b0n:s0 + b0n + b1n]))
            cols.append(_wrap16(dst16[s0:s0 + b0n + b1n]))
        idx = np.ascontiguousarray(np.concatenate(cols, axis=1))
        dofw = np.ascontiguousarray(dof.reshape(cfg.tt, P).T)
        return idx, dofw

    r2 = (src // nsh) * cfg.rowsd + src % nsh  # layer-2 table row per node id
    r2_s = r2[order]

    per_core = []
    for k in range(cfg.ncores):
        sl = slice(bounds[k], bounds[k + 1])
        sk, dk = src_s[sl], dst_s[sl]
        dloc = dk - k * nsh
        a = build_layer(sk, dloc, sk)
        b = build_layer(sk, dloc, r2_s[sl])
        if a is None or b is None:
            return None
        xTd = np.zeros((IN + 1, cfg.rowsd), np.float16)
        xTd[:IN, :nsh] = np.asarray(
            x[k * nsh:(k + 1) * nsh], np.float32).T.astype(np.float16)
        xTd[IN, :] = 1.0
        per_core.append(dict(idxA=a[0], doffA=a[1], idxB=b[0], doffB=b[1],
                             xTd=xTd))

    xT = np.zeros((IN + 1, cfg.rows1), np.float16)
    xT[:IN, :n] = np.asarray(x, np.float32).T.astype(np.float16)
    xT[IN, :] = 1.0
    w1c = np.concatenate([np.asarray(W1l), np.asarray(W1r)], axis=0)
    w1cb = np.concatenate([np.asarray(b1l), np.asarray(b1r)])[None, :]
    w1c_h = np.concatenate([w1c.T, w1cb], axis=0).astype(np.float16)
    w2c = np.concatenate([np.asarray(W2l), np.asarray(W2r)], axis=0)
    w2c_h = np.ascontiguousarray(w2c.T).astype(np.float16)
    b2c_h = np.concatenate([np.asarray(b2l), np.asarray(b2r)])[None, :].astype(
        np.float32)
    att1_h = np.asarray(att1, np.float32).reshape(1, D1).astype(np.float16)
    att2_h = np.asarray(att2, np.float32).reshape(1, D2).astype(np.float16)
    b1row_h = np.asarray(bias1, np.float32).reshape(1, D1)

    shared = dict(xT=xT, w1c=w1c_h, w2c=w2c_h, b2c=b2c_h, att1=att1_h,
                  att2=att2_h, b1row=b1row_h)
    return [dict(shared, **pc) for pc in per_core]


def assemble_output(results, bias2, cfg: Cfg):
    outs = []
    b2 = np.asarray(bias2, np.float32)
    for k in range(cfg.ncores):
        arr = results[k]["out_raw"][:cfg.nsh]  # [nsh, 40] node-major
        num = arr[:, :D2].reshape(cfg.nsh, H2, OUT)
        den = arr[:, D2:D2 + H2]
        outk = (num / (den[:, :, None] + 1e-16)).mean(axis=1) + b2[None, :]
        outs.append(outk.astype(np.float32))
    return np.concatenate(outs, axis=0)


# ---------------------------------------------------------------- fallback
def _reference_numpy(x, edge_index, W1l, b1l, W1r, b1r, att1, bias1,
                     W2l, b2l, W2r, b2r, att2, bias2):
    def gatv2(x, src, dst, Wl, bl, Wr, br, att, bias, concat):
        n = x.shape[0]
        H, C = att.shape
        xl = (x @ Wl.T + bl).reshape(n, H, C)
        xr = (x @ Wr.T + br).reshape(n, H, C)
        ee = xl[src] + xr[dst]
        ee = np.where(ee > 0, ee, NEG_ATT * ee)
        logits = np.einsum("ehc,hc->eh", ee, att)
        m = np.full((n, H), -np.inf, np.float32)
        np.maximum.at(m, dst, logits)
        m = np.where(np.isfinite(m), m, 0.0)
        p = np.exp(logits - m[dst])
        den = np.zeros((n, H), np.float32)
        np.add.at(den, dst, p)
        alpha = p / (den[dst] + 1e-16)
        out = np.zeros((n, H, C), np.float32)
        np.add.at(out, dst, alpha[..., None] * xl[src])
        if concat:
            return out.reshape(n, H * C) + bias
        return out.mean(axis=1) + bias

    src, dst = edge_index[0].astype(np.int64), edge_index[1].astype(np.int64)
    h = gatv2(np.asarray(x, np.float32), src, dst, W1l, b1l, W1r, b1r, att1,
              bias1, True)
    h = np.where(h > 0, h, NEG_ACT * h)
    return gatv2(h, src, dst, W2l, b2l, W2r, b2r, att2, bias2, False)


# ---------------------------------------------------------------- entry point
@functools.lru_cache(maxsize=1)
def _compiled():
    return build_program(CFG)


_LAST_RESULTS = {}


def kernel(x, edge_index, W1l, b1l, W1r, b1r, att1, bias1,
           W2l, b2l, W2r, b2r, att2, bias2):
    args = (x, edge_index, W1l, b1l, W1r, b1r, att1, bias1,
            W2l, b2l, W2r, b2r, att2, bias2)
    if (np.asarray(x).shape != (N, IN)
            or np.asarray(edge_index).shape != (2, E)):
        return _reference_numpy(*[np.asarray(a, np.float32) if i != 1 else
                                  np.asarray(a) for i, a in enumerate(args)])

    in_maps = host_prep(*args, CFG)
    if in_maps is None:
        return _reference_numpy(*[np.asarray(a, np.float32) if i != 1 else
                                  np.asarray(a) for i, a in enumerate(args)])

    from concourse.bass_utils import run_bass_kernel_spmd
    nc = _compiled()
    res = run_bass_kernel_spmd(nc, in_maps, core_ids=list(range(NCORES)),
                               trace=False)
    _LAST_RESULTS["res"] = res
    return assemble_output(res.results, bias2, CFG)


# revision 17
# speedup vs baseline: 2.3615x; 1.0404x over previous
"""GATv2 (2-layer, PyG semantics) on 8 Trainium2 NeuronCores.

Strategy (graph/data parallel, dst-sharded):
  - Nodes sharded by destination range across 8 cores (12500 nodes/core).
  - Node tables have 256-B rows [xl_n | xr_n]; per-edge endpoint features
    are fetched with bulk dma_gather (one SWDGE call per shard per group
    of 4 windows, ~0.34ns/descriptor) instead of per-tile indirect DMAs
    (~1us fixed cost each). int16 gather indices are made to fit via a
    signed +/-32K base trick (two shard calls cover 100352 rows).
  - Dst-side rows are gathered from core-LOCAL tables (t1d/t2) with
    group-relative indices so the SPMD program stays core-independent.
  - Edge softmax math on DVE/ACT per region; aggregation via one-hot
    matmuls (stationary one-hot, streamed weights -> node-major agg).
  - Window epilogue is node-major; h is transposed once per window on PE
    to feed the layer-2 dense matmul; raw [num|den] go back to the host
    which finishes divide/mean/bias.
"""

import functools
import sys

import numpy as np

sys.path.insert(0, "/opt/trn_rl_repo")

# ---------------------------------------------------------------- constants
N = 100_000
E = 1_600_000
IN = 9
HID = 16
H1 = 4
H2 = 4
OUT = 9
D1 = H1 * HID  # 64
D2 = H2 * OUT  # 36
NEG_ATT = 0.2
NEG_ACT = 0.01
NCORES = 8
NSH = N // NCORES  # 12500 nodes per core
WIN = 128  # dst nodes per window
P = 128
GW1 = 64  # gather elems layer 1 (fp16) -> 128B
GW2 = 40  # gather elems layer 2 (fp16) -> 80B
RW = 128  # table row width (fp16) -> 256B stride
B0C = 1536  # slots per window, shard block 0 (12 tiles)
B1C = 896  # slots per window, shard block 1 (7 tiles)
B0T = B0C // P
B1T = B1C // P
TWS = B0T + B1T  # 19 tiles per window
SPW = B0C + B1C  # 2432 slots per window
G = 4  # windows per gather group
SHARD = 65536  # shard-0 row threshold


class Cfg:
    def __init__(self, n=N, e=E, ncores=NCORES, dense_chunks=28):
        self.n = n
        self.e = e
        self.ncores = ncores
        self.nsh = n // ncores
        self.nw = -(-self.nsh // WIN)  # 98 windows per core
        self.tt = self.nw * TWS
        self.nslots = self.tt * P
        self.rows1 = ((n + 1023) // 1024) * 1024  # 100352
        self.rowsd = self.nw * WIN  # 12544 local rows
        self.rows2 = self.rowsd * ncores  # 100352
        self.dense_tiles = self.rows1 // P
        self.dense_chunks = dense_chunks
        assert self.dense_tiles % dense_chunks == 0
        self.chunk_tiles = self.dense_tiles // dense_chunks
        self.wb = 7 if self.chunk_tiles % 7 == 0 else 1
        assert self.chunk_tiles % self.wb == 0
        # groups of windows sharing gather calls
        self.groups = []
        w = 0
        while w < self.nw:
            g = min(G, self.nw - w)
            self.groups.append((w, g))
            w += g
        # idx columns per group: (B0C + B1C + SPW) * nwg / 16
        self.gcols = [(B0C + B1C + SPW) * g // 16 for _, g in self.groups]
        self.gcol_off = np.concatenate([[0], np.cumsum(self.gcols)])
        self.idxcols = int(self.gcol_off[-1])
        self.gtile_off = np.concatenate(
            [[0], np.cumsum([TWS * g for _, g in self.groups])])


CFG = Cfg()


# ---------------------------------------------------------------- device code
def build_program(cfg: Cfg):
    """Build the SPMD single-core Bass program (same NEFF on all cores)."""
    import concourse.bacc as bacc
    import concourse.tile as tile
    from concourse import mybir
    from concourse.ap_utils import ap_is_contiguous

    f32 = mybir.dt.float32
    f16 = mybir.dt.float16
    i16 = mybir.dt.int16
    AF = mybir.ActivationFunctionType
    ALU = mybir.AluOpType

    nc = bacc.Bacc("TRN2", target_bir_lowering=False, debug=False,
                   num_devices=cfg.ncores, num_swdge_queues=4)

    def dma_gather_raw(out_ap, in_ap, idxs_ap, num_idxs, elem_size,
                       elem_step=RW, queue_num=0):
        """dma_gather minus the %256 elem-size assert (ucode only needs the
        row STRIDE to be a multiple of 256B; validated on HW)."""
        g = nc.gpsimd
        assert in_ap.ap[0][0] == elem_step
        stride_256, rem = divmod(elem_step * 2, 256)
        assert rem == 0 and stride_256 < 256
        assert ap_is_contiguous(out_ap.ap[1:])
        assert ap_is_contiguous(idxs_ap.ap[1:])
        assert idxs_ap.dtype == i16
        _in_ap = g.lower_ap_dma(in_ap, for_custom_bir_dma=True)
        _idxs_ap = g.lower_ap(idxs_ap)
        _out_ap = g.lower_ap(out_ap)
        return g.add_instruction(
            mybir.InstDMAGatherAnt(
                name=nc.get_next_instruction_name(),
                ins=[*_in_ap, _idxs_ap, g.lower_val_access(g.to_reg(num_idxs))],
                outs=[_out_ap],
                transpose=False, num_idxs=num_idxs, elem_size=elem_size,
                stride_bytes_256=stride_256, gen_mode=0, single_packet=False,
                queue_num=queue_num, sbuf_tokens_per_rank=0,
                sbuf_free_dim_per_rank=0,
                sbuf_free_dim_pad_per_rank=0, sbuf_byte_offset=0))

    # ---------------- dram I/O
    xT = nc.dram_tensor("xT", [IN + 1, cfg.rows1], f16, kind="ExternalInput")
    xTd = nc.dram_tensor("xTd", [IN + 1, cfg.rowsd], f16,
                         kind="ExternalInput")
    w1c = nc.dram_tensor("w1c", [IN + 1, 2 * D1], f16, kind="ExternalInput")
    w2c = nc.dram_tensor("w2c", [D1, 2 * D2], f16, kind="ExternalInput")
    b2c = nc.dram_tensor("b2c", [1, 2 * D2], f32, kind="ExternalInput")
    att1 = nc.dram_tensor("att1", [1, D1], f16, kind="ExternalInput")
    att2 = nc.dram_tensor("att2", [1, D2], f16, kind="ExternalInput")
    b1row = nc.dram_tensor("b1row", [1, D1], f32, kind="ExternalInput")
    idxA = nc.dram_tensor("idxA", [P, cfg.idxcols], i16, kind="ExternalInput")
    idxB = nc.dram_tensor("idxB", [P, cfg.idxcols], i16, kind="ExternalInput")
    doffA = nc.dram_tensor("doffA", [P, cfg.tt], f32, kind="ExternalInput")
    doffB = nc.dram_tensor("doffB", [P, cfg.tt], f32, kind="ExternalInput")
    out_raw = nc.dram_tensor("out_raw", [cfg.rowsd, D2 + H2], f32,
                             kind="ExternalOutput")

    t1 = nc.dram_tensor("t1", [cfg.rows1, RW], f16)
    t1d = nc.dram_tensor("t1d", [cfg.rowsd, RW], f16)
    t2 = nc.dram_tensor("t2", [cfg.rowsd, RW], f16)
    t2g = nc.dram_tensor("t2g", [cfg.rows2, RW], f16, addr_space="Shared")

    OC1 = D1 + H1  # 68
    OC2 = D2 + H2  # 40
    MAXT0 = G * B0T  # 48 tiles, B0 region of a full group
    MAXT1 = G * B1T  # 28

    with tile.TileContext(nc) as tc:
        import contextlib
        ctx = contextlib.ExitStack()
        with ctx:
            consts = ctx.enter_context(tc.tile_pool(name="consts", bufs=1))
            dofp = ctx.enter_context(tc.tile_pool(name="dofp", bufs=1))
            idxp = ctx.enter_context(tc.tile_pool(name="idxp", bufs=2))
            xtp = ctx.enter_context(tc.tile_pool(name="xtp", bufs=2))
            stage = ctx.enter_context(tc.tile_pool(name="stage", bufs=3))
            gath = ctx.enter_context(tc.tile_pool(name="gath", bufs=2))
            emath = ctx.enter_context(tc.tile_pool(name="emath", bufs=2))
            ohp = ctx.enter_context(tc.tile_pool(name="ohp", bufs=2))
            wtp = ctx.enter_context(tc.tile_pool(name="wtp", bufs=2))
            epi = ctx.enter_context(tc.tile_pool(name="epi", bufs=2))
            psa = ctx.enter_context(tc.tile_pool(name="psa", bufs=1,
                                                 space="PSUM"))
            psd = ctx.enter_context(tc.tile_pool(name="psd", bufs=2,
                                                 space="PSUM"))
            pse = ctx.enter_context(tc.tile_pool(name="pse", bufs=1,
                                                 space="PSUM"))

            # ---------------- constants
            w1c_sb = consts.tile([IN + 1, 2 * D1], f16)
            nc.sync.dma_start(out=w1c_sb[:], in_=w1c.ap())
            w2c_sb = consts.tile([D1, 2 * D2], f16)
            nc.sync.dma_start(out=w2c_sb[:], in_=w2c.ap())
            att1_sb = consts.tile([P, D1], f16)
            nc.sync.dma_start(out=att1_sb[0:1, :], in_=att1.ap())
            nc.gpsimd.partition_broadcast(att1_sb[:], att1_sb[0:1, :])
            att2_sb = consts.tile([P, D2], f16)
            nc.sync.dma_start(out=att2_sb[0:1, :], in_=att2.ap())
            nc.gpsimd.partition_broadcast(att2_sb[:], att2_sb[0:1, :])
            b2rep = consts.tile([P, 2 * D2], f32)
            nc.sync.dma_start(out=b2rep[0:1, :], in_=b2c.ap())
            nc.gpsimd.partition_broadcast(b2rep[:], b2rep[0:1, :])
            b1rep = consts.tile([P, D1], f32)
            nc.sync.dma_start(out=b1rep[0:1, :], in_=b1row.ap())
            nc.gpsimd.partition_broadcast(b1rep[:], b1rep[0:1, :])
            iota_i = consts.tile([P, P], mybir.dt.int32)
            nc.gpsimd.iota(iota_i[:], pattern=[[1, P]], base=0,
                           channel_multiplier=0)
            iota_f = consts.tile([P, P], f32)
            nc.vector.tensor_copy(out=iota_f[:], in_=iota_i[:])
            iota_p = consts.tile([P, P], mybir.dt.int32)
            nc.gpsimd.iota(iota_p[:], pattern=[[0, P]], base=0,
                           channel_multiplier=1)
            iota_pf = consts.tile([P, P], f32)
            nc.vector.tensor_copy(out=iota_pf[:], in_=iota_p[:])
            ident = consts.tile([P, P], f16)
            nc.vector.tensor_tensor(out=ident[:], in0=iota_f[:],
                                    in1=iota_pf[:], op=ALU.is_equal)

            doffA_sb = dofp.tile([P, cfg.tt], f32)
            nc.sync.dma_start(out=doffA_sb[:], in_=doffA.ap())
            doffB_sb = dofp.tile([P, cfg.tt], f32)
            nc.sync.dma_start(out=doffB_sb[:], in_=doffB.ap())

            # ---------------- phase 1: dense node tables (t1 global, t1d loc)
            def dense_pass(src_dram, dst_dram, ntiles, nchunks):
                ck = ntiles // nchunks
                wb = 7 if ck % 7 == 0 else 1
                tv = dst_dram.ap().rearrange("(b t n) f -> b n t f",
                                             t=wb, n=P)
                for c in range(nchunks):
                    xt_sb = xtp.tile([IN + 1, cfg.chunk_tiles * P], f16,
                                     tag="xt")
                    nc.sync.dma_start(
                        out=xt_sb[:, 0:ck * P],
                        in_=src_dram.ap()[:, c * ck * P:(c + 1) * ck * P])
                    for b in range(ck // wb):
                        st = stage.tile([P, wb, 2 * D1], f16, tag="st")
                        for j in range(wb):
                            t = b * wb + j
                            mm = psd.tile([P, 2 * D1], f32, tag="mm")
                            nc.tensor.matmul(out=mm[:],
                                             lhsT=xt_sb[:, t * P:(t + 1) * P],
                                             rhs=w1c_sb[:], start=True,
                                             stop=True)
                            nc.scalar.copy(out=st[:, j, :], in_=mm[:])
                        nc.sync.dma_start(out=tv[c * (ck // wb) + b],
                                          in_=st[:])

            dense_pass(xT, t1, cfg.dense_tiles, cfg.dense_chunks)
            dense_pass(xTd, t1d, cfg.rowsd // P, 7)

            tc.strict_bb_all_engine_barrier()

            # ---------------- edge phase builder
            def edge_layer(layer):
                if layer == 1:
                    D, H, C, OC, GW = D1, H1, HID, OC1, GW1
                    tsrc, tdst = t1, t1d
                    idxL, doff_sb, att_sb = idxA, doffA_sb, att1_sb
                else:
                    D, H, C, OC, GW = D2, H2, OUT, OC2, GW2
                    tsrc, tdst = t2g, t2
                    idxL, doff_sb, att_sb = idxB, doffB_sb, att2_sb

                for gi, (w0, nwg) in enumerate(cfg.groups):
                    t0 = nwg * B0T  # B0 tiles in this group
                    t1n = nwg * B1T
                    tg = t0 + t1n
                    c0 = nwg * B0C // 16  # idx cols per call
                    c1 = nwg * B1C // 16
                    cd = nwg * SPW // 16
                    cbase = int(cfg.gcol_off[gi])
                    tbase = int(cfg.gtile_off[gi])

                    idx_sb = idxp.tile([P, (B0C + B1C + SPW) * G // 16], i16,
                                       tag="idx")
                    nc.sync.dma_start(
                        out=idx_sb[:, 0:c0 + c1 + cd],
                        in_=idxL.ap()[:, cbase:cbase + c0 + c1 + cd])

                    xgs = gath.tile([P, MAXT0 + MAXT1, GW], f16, tag="xgs")
                    xgd = gath.tile([P, MAXT0 + MAXT1, GW], f16, tag="xgd")
                    # balanced split: each queue gets nwg*SPW/2 idxs per group
                    h0 = (nwg // 2) * B0T  # b0 split at window boundary
                    h1 = (nwg // 2) * B1T
                    hd = tg // 2
                    base0 = tsrc.ap()[32768:32769, 0:GW]
                    base1 = tsrc.ap()[98304:98305, 0:GW]
                    based = tdst.ap()[w0 * WIN:w0 * WIN + 1, 64:64 + GW]
                    ic0 = h0 * P // 16
                    ic1 = h1 * P // 16
                    icd = hd * P // 16
                    dma_gather_raw(xgs[:, 0:h0, :], base0,
                                   idx_sb[:, 0:ic0], h0 * P, GW, queue_num=0)
                    dma_gather_raw(xgs[:, h0:t0, :], base0,
                                   idx_sb[:, ic0:c0], (t0 - h0) * P, GW,
                                   queue_num=1)
                    dma_gather_raw(xgs[:, t0:t0 + h1, :], base1,
                                   idx_sb[:, c0:c0 + ic1], h1 * P, GW,
                                   queue_num=0)
                    dma_gather_raw(xgs[:, t0 + h1:tg, :], base1,
                                   idx_sb[:, c0 + ic1:c0 + c1],
                                   (t1n - h1) * P, GW, queue_num=1)
                    dma_gather_raw(xgd[:, 0:hd, :], based,
                                   idx_sb[:, c0 + c1:c0 + c1 + icd],
                                   hd * P, GW, queue_num=2)
                    dma_gather_raw(xgd[:, hd:tg, :], based,
                                   idx_sb[:, c0 + c1 + icd:c0 + c1 + cd],
                                   (tg - hd) * P, GW, queue_num=3)

                    oh = ohp.tile([P, MAXT0 + MAXT1, P], f16, tag="oh")
                    epre = emath.tile([P, MAXT0 + MAXT1, D], f16, tag="epre")
                    ee = emath.tile([P, MAXT0 + MAXT1, D], f16, tag="ee")
                    tmp = emath.tile([P, MAXT0 + MAXT1, D], f16, tag="tmp")
                    logits = emath.tile([P, (MAXT0 + MAXT1) * H], f32,
                                        tag="logits")
                    pp = emath.tile([P, (MAXT0 + MAXT1) * H], f32, tag="pp")
                    wt = wtp.tile([P, MAXT0 + MAXT1, OC], f16, tag="wt")

                    nc.vector.tensor_tensor(
                        out=oh[:, 0:tg, :],
                        in0=iota_f[:].unsqueeze(1).to_broadcast([P, tg, P]),
                        in1=doff_sb[:, tbase:tbase + tg].unsqueeze(
                            2).to_broadcast([P, tg, P]),
                        op=ALU.is_equal)
                    nc.vector.tensor_tensor(out=epre[:, 0:tg, :],
                                            in0=xgs[:, 0:tg, 0:D],
                                            in1=xgd[:, 0:tg, 0:D],
                                            op=ALU.add)
                    nc.vector.scalar_tensor_tensor(
                        out=ee[:, 0:tg, :], in0=epre[:, 0:tg, :],
                        scalar=NEG_ATT, in1=epre[:, 0:tg, :],
                        op0=ALU.mult, op1=ALU.max)
                    nc.vector.tensor_tensor(
                        out=tmp[:, 0:tg, :], in0=ee[:, 0:tg, :],
                        in1=att_sb[:, 0:D].unsqueeze(1).to_broadcast(
                            [P, tg, D]),
                        op=ALU.mult)
                    nc.vector.tensor_reduce(
                        out=logits[:, 0:tg * H],
                        in_=tmp[:, 0:tg, :].rearrange(
                            "p t (h c) -> p (t h) c", c=C),
                        axis=mybir.AxisListType.X, op=ALU.add)
                    nc.scalar.activation(out=pp[:, 0:tg * H],
                                         in_=logits[:, 0:tg * H], func=AF.Exp)
                    nc.vector.tensor_copy(
                        out=wt[:, 0:tg, D:OC],
                        in_=pp[:, 0:tg * H].rearrange("p (t h) -> p t h",
                                                      h=H))
                    nc.vector.tensor_tensor(
                        out=wt[:, 0:tg, 0:D].rearrange(
                            "p t (h c) -> p t h c", c=C),
                        in0=xgs[:, 0:tg, 0:D].rearrange(
                            "p t (h c) -> p t h c", c=C),
                        in1=wt[:, 0:tg, D:OC].unsqueeze(3).to_broadcast(
                            [P, tg, H, C]),
                        op=ALU.mult)

                    for wi in range(nwg):
                        w = w0 + wi
                        # node-major agg: lhsT = one-hot (stationary)
                        agg = psa.tile([P, OC], f32,
                                       tag=f"agg{wi % 2}")
                        tiles = ([wi * B0T + j for j in range(B0T)] +
                                 [t0 + wi * B1T + j for j in range(B1T)])
                        for i, t in enumerate(tiles):
                            nc.tensor.matmul(out=agg[:], lhsT=oh[:, t, :],
                                             rhs=wt[:, t, :], start=(i == 0),
                                             stop=(i == len(tiles) - 1))

                        if layer == 1:
                            den = epi.tile([P, H], f32, tag="den")
                            nc.vector.tensor_scalar(
                                den[:], agg[:, D:OC], 1e-16, None,
                                op0=ALU.add)
                            rec = epi.tile([P, H], f32, tag="rec")
                            nc.vector.reciprocal(out=rec[:], in_=den[:])
                            hpre = epi.tile([P, D1], f32, tag="hpre")
                            nc.vector.tensor_tensor(
                                out=hpre[:].rearrange("p (h c) -> p h c",
                                                      c=HID),
                                in0=agg[:, 0:D1].rearrange(
                                    "p (h c) -> p h c", c=HID),
                                in1=rec[:].unsqueeze(2).to_broadcast(
                                    [P, H1, HID]),
                                op=ALU.mult)
                            hb = epi.tile([P, D1], f32, tag="hb")
                            nc.vector.tensor_tensor(out=hb[:], in0=hpre[:],
                                                    in1=b1rep[:], op=ALU.add)
                            h16 = epi.tile([P, D1], f16, tag="h16")
                            nc.vector.scalar_tensor_tensor(
                                out=h16[:], in0=hb[:], scalar=NEG_ACT,
                                in1=hb[:], op0=ALU.mult, op1=ALU.max)
                            hTp = pse.tile([D1, P], f16, tag="hTp")
                            nc.tensor.transpose(out=hTp[:], in_=h16[:],
                                                identity=ident[:])
                            hT = epi.tile([D1, P], f16, tag="hT")
                            nc.scalar.copy(out=hT[:], in_=hTp[:])
                            t2mm = pse.tile([P, 2 * D2], f32, tag="t2mm")
                            nc.tensor.matmul(out=t2mm[:], lhsT=hT[:],
                                             rhs=w2c_sb[:], start=True,
                                             stop=True)
                            st2 = epi.tile([P, RW], f16, tag="st2")
                            nc.vector.memset(st2[:], 0)
                            nc.vector.scalar_tensor_tensor(
                                out=st2[:, 0:D2], in0=t2mm[:, 0:D2],
                                scalar=1.0, in1=b2rep[:, 0:D2],
                                op0=ALU.mult, op1=ALU.add)
                            nc.vector.scalar_tensor_tensor(
                                out=st2[:, 64:64 + D2],
                                in0=t2mm[:, D2:2 * D2], scalar=1.0,
                                in1=b2rep[:, D2:2 * D2],
                                op0=ALU.mult, op1=ALU.add)
                            nc.sync.dma_start(
                                out=t2.ap()[w * P:(w + 1) * P, :],
                                in_=st2[:])
                        else:
                            cp = epi.tile([P, OC2], f32, tag="cp")
                            nc.scalar.copy(out=cp[:], in_=agg[:])
                            nc.sync.dma_start(
                                out=out_raw.ap()[w * P:(w + 1) * P, :],
                                in_=cp[:])

            # ---------------- phase 2: layer-1 edges
            edge_layer(1)
            tc.strict_bb_all_engine_barrier()

            # ---------------- phase 3: allgather layer-2 table
            nc.gpsimd.collective_compute(
                "AllGather", mybir.AluOpType.bypass,
                replica_groups=[list(range(cfg.ncores))],
                ins=[t2.ap()], outs=[t2g.ap()])
            tc.strict_bb_all_engine_barrier()

            # ---------------- phase 4: layer-2 edges
            edge_layer(2)

    nc.compile()
    return nc


# ---------------------------------------------------------------- host prep
def _wrap16(v):
    """[n] -> [128, n/16]: index j at [j%16, j//16], replicated x8."""
    a = v.reshape(-1, 16).T  # [16, n/16]
    return np.tile(a, (8, 1))


def host_prep(x, edge_index, W1l, b1l, W1r, b1r, att1, bias1,
              W2l, b2l, W2r, b2r, att2, bias2, cfg: Cfg):
    """Numpy-only preprocessing: per-layer slot layouts + weight layouts."""
    n, nsh = cfg.n, cfg.nsh
    src = np.asarray(edge_index[0], dtype=np.int64)
    dst = np.asarray(edge_index[1], dtype=np.int64)

    order = np.argsort(dst, kind="stable")
    src_s, dst_s = src[order], dst[order]
    bounds = np.searchsorted(dst_s, np.arange(cfg.ncores + 1) * nsh)

    ngr = len(cfg.groups)
    grp_sizes = np.array([g for _, g in cfg.groups], np.int64)
    grp_slot_base = np.concatenate([[0], np.cumsum(grp_sizes * SPW)])
    grp_of_w = np.repeat(np.arange(ngr), grp_sizes)
    wi_of_w = np.concatenate([np.arange(g) for g in grp_sizes])
    w0_of_g = np.array([w for w, _ in cfg.groups], np.int64)

    def build_layer(sk, dloc, rowv):
        """Slot layout for one layer. rowv = table row of src per edge."""
        win = dloc >> 7
        blk = (rowv >= SHARD).astype(np.int64)
        key = win * 2 + blk
        ord2 = np.argsort(key, kind="stable")
        ks = key[ord2]
        cnt = np.bincount(ks, minlength=cfg.nw * 2)
        if (cnt[0::2] > B0C - 1).any() or (cnt[1::2] > B1C - 1).any():
            return None
        starts = np.concatenate([[0], np.cumsum(cnt)])[:-1]
        within = np.arange(len(ks)) - starts[ks]
        wv, bv = ks >> 1, ks & 1
        gv = grp_of_w[wv]
        wiv = wi_of_w[wv]
        nwgv = grp_sizes[gv]
        base = grp_slot_base[gv] + np.where(
            bv == 0, wiv * B0C, nwgv * B0C + wiv * B1C)
        slot = np.empty(len(ks), np.int64)
        slot[ord2] = base + within

        src16 = np.zeros(cfg.nslots, np.int16)
        src16[slot] = (rowv - np.where(blk == 1, 98304, 32768)).astype(
            np.int16)
        dst16 = np.zeros(cfg.nslots, np.int16)
        dst16[slot] = (dloc - w0_of_g[grp_of_w[win]] * WIN).astype(np.int16)
        dof = np.full(cfg.nslots, -1.0, np.float32)
        dof[slot] = (dloc & 127).astype(np.float32)

        cols = []
        for gi, (w0g, nwg) in enumerate(cfg.groups):
            s0 = int(grp_slot_base[gi])
            b0n, b1n = nwg * B0C, nwg * B1C
            cols.append(_wrap16(src16[s0:s0 + b0n]))
            cols.append(_wrap16(src16[s0 + b0n:s0 + b0n + b1n]))
            cols.append(_wrap16(dst16[s0:s0 + b0n + b1n]))
        idx = np.ascontiguousarray(np.concatenate(cols, axis=1))
        dofw = np.ascontiguousarray(dof.reshape(cfg.tt, P).T)
        return idx, dofw

    r2 = (src // nsh) * cfg.rowsd + src % nsh  # layer-2 table row per node id
    r2_s = r2[order]

    per_core = []
    for k in range(cfg.ncores):
        sl = slice(bounds[k], bounds[k + 1])
        sk, dk = src_s[sl], dst_s[sl]
        dloc = dk - k * nsh
        a = build_layer(sk, dloc, sk)
        b = build_layer(sk, dloc, r2_s[sl])
        if a is None or b is None:
            return None
        xTd = np.zeros((IN + 1, cfg.rowsd), np.float16)
        xTd[:IN, :nsh] = np.asarray(
            x[k * nsh:(k + 1) * nsh], np.float32).T.astype(np.float16)
        xTd[IN, :] = 1.0
        per_core.append(dict(idxA=a[0], doffA=a[1], idxB=b[0], doffB=b[1],
                             xTd=xTd))

    xT = np.zeros((IN + 1, cfg.rows1), np.float16)
    xT[:IN, :n] = np.asarray(x, np.float32).T.astype(np.float16)
    xT[IN, :] = 1.0
    w1c = np.concatenate([np.asarray(W1l), np.asarray(W1r)], axis=0)
    w1cb = np.concatenate([np.asarray(b1l), np.asarray(b1r)])[None, :]
    w1c_h = np.concatenate([w1c.T, w1cb], axis=0).astype(np.float16)
    w2c = np.concatenate([np.asarray(W2l), np.asarray(W2r)], axis=0)
    w2c_h = np.ascontiguousarray(w2c.T).astype(np.float16)
    b2c_h = np.concatenate([np.asarray(b2l), np.asarray(b2r)])[None, :].astype(
        np.float32)
    att1_h = np.asarray(att1, np.float32).reshape(1, D1).astype(np.float16)
    att2_h = np.asarray(att2, np.float32).reshape(1, D2).astype(np.float16)
    b1row_h = np.asarray(bias1, np.float32).reshape(1, D1)

    shared = dict(xT=xT, w1c=w1c_h, w2c=w2c_h, b2c=b2c_h, att1=att1_h,
                  att2=att2_h, b1row=b1row_h)
    return [dict(shared, **pc) for pc in per_core]


def assemble_output(results, bias2, cfg: Cfg):
    outs = []
    b2 = np.asarray(bias2, np.float32)
    for k in range(cfg.ncores):
        arr = results[k]["out_raw"][:cfg.nsh]  # [nsh, 40] node-major
        num = arr[:, :D2].reshape(cfg.nsh, H2, OUT)
        den = arr[:, D2:D2 + H2]
        outk = (num / (den[:, :, None] + 1e-16)).mean(axis=1) + b2[None, :]
        outs.append(outk.astype(np.float32))
    return np.concatenate(outs, axis=0)


# ---------------------------------------------------------------- fallback
def _reference_numpy(x, edge_index, W1l, b1l, W1r, b1r, att1, bias1,
                     W2l, b2l, W2r, b2r, att2, bias2):
    def gatv2(x, src, dst, Wl, bl, Wr, br, att, bias, concat):
        n = x.shape[0]
        H, C = att.shape
        xl = (x @ Wl.T + bl).reshape(n, H, C)
        xr = (x @ Wr.T + br).reshape(n, H, C)
        ee = xl[src] + xr[dst]
        ee = np.where(ee > 0, ee, NEG_ATT * ee)
        logits = np.einsum("ehc,hc->eh", ee, att)
        m = np.full((n, H), -np.inf, np.float32)
        np.maximum.at(m, dst, logits)
        m = np.where(np.isfinite(m), m, 0.0)
        p = np.exp(logits - m[dst])
        den = np.zeros((n, H), np.float32)
        np.add.at(den, dst, p)
        alpha = p / (den[dst] + 1e-16)
        out = np.zeros((n, H, C), np.float32)
        np.add.at(out, dst, alpha[..., None] * xl[src])
        if concat:
            return out.reshape(n, H * C) + bias
        return out.mean(axis=1) + bias

    src, dst = edge_index[0].astype(np.int64), edge_index[1].astype(np.int64)
    h = gatv2(np.asarray(x, np.float32), src, dst, W1l, b1l, W1r, b1r, att1,
              bias1, True)
    h = np.where(h > 0, h, NEG_ACT * h)
    return gatv2(h, src, dst, W2l, b2l, W2r, b2r, att2, bias2, False)


# ---------------------------------------------------------------- entry point
@functools.lru_cache(maxsize=1)
def _compiled():
    return build_program(CFG)


_LAST_RESULTS = {}


def kernel(x, edge_index, W1l, b1l, W1r, b1r, att1, bias1,
           W2l, b2l, W2r, b2r, att2, bias2):
    args = (x, edge_index, W1l, b1l, W1r, b1r, att1, bias1,
            W2l, b2l, W2r, b2r, att2, bias2)
    if (np.asarray(x).shape != (N, IN)
            or np.asarray(edge_index).shape != (2, E)):
        return _reference_numpy(*[np.asarray(a, np.float32) if i != 1 else
                                  np.asarray(a) for i, a in enumerate(args)])

    in_maps = host_prep(*args, CFG)
    if in_maps is None:
        return _reference_numpy(*[np.asarray(a, np.float32) if i != 1 else
                                  np.asarray(a) for i, a in enumerate(args)])

    from concourse.bass_utils import run_bass_kernel_spmd
    nc = _compiled()
    res = run_bass_kernel_spmd(nc, in_maps, core_ids=list(range(NCORES)),
                               trace=False)
    _LAST_RESULTS["res"] = res
    return assemble_output(res.results, bias2, CFG)
